# revision 68
# baseline (speedup 1.0000x reference)
"""Trainium2 Bass kernel for nn_BasicTransDecoderBlock (dense_transformer).

Strategy: data-parallel over batch B=8 across 8 NeuronCores (1 sample/core).
BatchNorm batch statistics are synchronized with two small AllReduces.
The attention is softmax-free and reassociated:
    O = Q' (K'^T V')/d  +  (bias @ V')/d
which collapses the dominant QK^T/AV FLOPs into tiny per-head d x d matmuls.

v2 optimizations over the first working version:
 - Depthwise convs run on RAW (pre-BN) zero-padded inputs so they can
   execute during the BN-stats AllReduce flight; the BN affine commutes:
   DW(s*x+t) = s*DWraw(x) + t*kappa, applied afterwards on the Scalar
   engine with tiny per-edge border corrections (kappa deviates from the
   full 3x3 weight sum only at image borders).
 - BN statistics are computed with the Scalar engine's accum_out while
   building the padded bf16 images (one fused pass), not with DVE bn_stats.
 - Depthwise taps are split across DVE (scalar_tensor_tensor), Scalar
   (scaled shifted copies) and GpSimd (pair merges) -- STT is hard-capped
   at 1x on DVE so parallel engines beat more DVE work.
 - LayerNorm-q is algebraically deferred: normq_g folds into the Q
   pointwise weights on host, the per-(head,pixel) rs/mrs scalars apply
   AFTER the block-diagonal attention matmul, and BD@b_q pre-adds into
   the bias map.
 - Pad-region-only memsets on GpSimd; PSUM evacuations on Scalar; the
   upsampled residue accumulates directly in the to_out PSUM tile.

Self-contained: hardcodes all shapes; imports only the concourse runtime
shipped in the container.
"""
import sys
import numpy as np
import ml_dtypes

for _p in ("/opt/trn_rl_repo", "/root/.axon_site/_ro/trn_rl_repo"):
    if _p not in sys.path:
        sys.path.insert(0, _p)

import concourse.bass as bass
import concourse.bacc as bacc
import concourse.tile as tile
from concourse import mybir
from concourse.bass_utils import run_bass_kernel_spmd

FP32 = mybir.dt.float32
BF16 = mybir.dt.bfloat16
ALU = mybir.AluOpType
ACTF = mybir.ActivationFunctionType

B, IN_CH, OUT_CH, HEADS, DIM_HEAD, R = 8, 512, 256, 8, 32, 16
H1, W1, H2, W2 = 32, 32, 64, 64
EPS_BN, EPS_LN = 1e-5, 1e-6
N1, N2, NS = H1 * W1, H2 * W2, R * R     # 1024, 4096, 256
P = 128
NCORES = 8
PW1, PW2 = W1 + 2, W2 + 2                # padded widths 34, 66
PAD1, PAD2 = (H1 + 2) * PW1, (H2 + 2) * PW2   # 1156, 4356
TAPS = [(dy, dx) for dy in range(3) for dx in range(3)]


# ---------------------------------------------------------------- host helpers

def _interp_matrix(n_in, n_out):
    A = np.zeros((n_out, n_in), np.float32)
    xs = np.linspace(0.0, n_in - 1.0, n_out)
    for i, x in enumerate(xs):
        x0 = int(np.floor(x)); x1 = min(x0 + 1, n_in - 1)
        w = x - x0
        A[i, x0] += 1.0 - w
        A[i, x1] += w
    return A


def _head_major_perm():
    perm = np.zeros(OUT_CH, np.int64)
    for h in range(HEADS):
        for d in range(DIM_HEAD):
            perm[h * DIM_HEAD + d] = d * HEADS + h
    return perm


def _rel_bias_small(rel_table):
    c = np.stack(np.meshgrid(np.arange(R), np.arange(R), indexing="ij")).reshape(2, -1)
    rel = (c[:, :, None] - c[:, None, :]).transpose(1, 2, 0)
    rel[:, :, 0] += R - 1
    rel[:, :, 1] += R - 1
    rel[:, :, 0] *= 2 * R - 1
    idx = rel.sum(-1).reshape(-1)
    return np.asarray(rel_table, np.float32)[idx].reshape(NS, NS, HEADS)


def _r64_chunks():
    """Residue resize (32->64), ch-major: per 512-pixel output chunk only a
    few 128-pixel input tiles contribute."""
    Ay, Ax = _interp_matrix(H1, H2), _interp_matrix(W1, W2)
    R64 = np.kron(Ay, Ax).astype(np.float32)       # [4096, 1024]
    ktiles, blocks = [], []
    for nn in range(8):
        rows = R64[nn * 512:(nn + 1) * 512]
        used = [kk for kk in range(8)
                if np.abs(rows[:, kk * 128:(kk + 1) * 128]).sum() > 0]
        ktiles.append(used)
        for kk in used:
            blocks.append(rows[:, kk * 128:(kk + 1) * 128].T.copy())
    return ktiles, np.concatenate(blocks, axis=0)


_R64_KTILES, _R64_PACKED = _r64_chunks()
_N_R64_SLOTS = sum(len(k) for k in _R64_KTILES)


def _dw_ext(dw):
    """Extend a [C, 9] depthwise tap table with derived columns:
    col 9: W9 (full sum), 10: r0 (top row sum), 11: r2 (bottom row),
    12: c0 (left col), 13: c2 (right col),
    14-17: corner taps w00, w02, w20, w22."""
    C = dw.shape[0]
    e = np.zeros((C, 18), np.float32)
    e[:, 0:9] = dw
    e[:, 9] = dw.sum(1)
    e[:, 10] = dw[:, 0] + dw[:, 1] + dw[:, 2]
    e[:, 11] = dw[:, 6] + dw[:, 7] + dw[:, 8]
    e[:, 12] = dw[:, 0] + dw[:, 3] + dw[:, 6]
    e[:, 13] = dw[:, 2] + dw[:, 5] + dw[:, 8]
    e[:, 14] = dw[:, 0]
    e[:, 15] = dw[:, 2]
    e[:, 16] = dw[:, 6]
    e[:, 17] = dw[:, 8]
    return e


def _dw_diag(dw1, dwq, dwo):
    """Diagonal-weight matrices for the PE-side taps 1..4 of all three
    depthwise convs. Slot layout: DW1 tiles 0-3 (slots 0-15), DWQ tiles
    0-1 (16-23), DWO tiles 0-1 (24-31): [32*128, 128]."""
    out = np.zeros((32, P, P), np.float32)
    s = 0
    for tbl, ntiles in ((dw1, 4), (dwq, 2), (dwo, 2)):
        for tt in range(ntiles):
            for j in range(4):
                np.fill_diagonal(out[s], tbl[tt * P:(tt + 1) * P, 1 + j])
                s += 1
    return out.reshape(32 * P, P)


def _host_prep(inp):
    perm = _head_major_perm()
    f32 = lambda a: np.ascontiguousarray(np.asarray(a, np.float32))
    bf = lambda a: np.ascontiguousarray(np.asarray(a, np.float32).astype(ml_dtypes.bfloat16))

    kvw = np.asarray(inp["to_kv_pw"], np.float32).reshape(2 * OUT_CH, IN_CH)
    selb = np.zeros((16 * P, OUT_CH), np.float32)
    hh = np.arange(OUT_CH) // DIM_HEAD
    for blk in range(16):
        for h in range(HEADS):
            # stats row packing (set by the relayout DMA stream order):
            # row = 16*(blk//2) + 2h + (blk%2)
            selb[blk * P + 16 * (blk // 2) + 2 * h + (blk % 2), :] = (hh == h)

    gq = np.asarray(inp["normq_g"], np.float32).reshape(-1)   # head-major (h,d)
    wq = np.asarray(inp["to_q_pw"], np.float32).reshape(OUT_CH, OUT_CH)[perm].T
    wq = wq * gq[None, :]                                     # fold g into WQ
    selq1 = np.equal(hh[:, None], np.arange(HEADS)[None, :]).astype(np.float32)
    selq1 = selq1 / gq[:, None]
    selq2 = selq1 / gq[:, None]

    d = {
        "wch": f32(np.asarray(inp["conv_ch_w"], np.float32).reshape(OUT_CH, IN_CH).T),
        "wkv": bf(np.concatenate([kvw[perm].T, kvw[OUT_CH + perm].T], axis=1)),
        "wq": bf(wq),
        "wout": bf(np.asarray(inp["to_out_pw"], np.float32).reshape(OUT_CH, OUT_CH)[:, perm].T),
        "wmlp": bf(np.asarray(inp["mlp_w"], np.float32).reshape(OUT_CH, OUT_CH).T),
        "dw1w": f32(_dw_ext(np.asarray(inp["to_kv_dw"], np.float32).reshape(IN_CH, 9))),
        "dwqw": f32(_dw_ext(np.asarray(inp["to_q_dw"], np.float32).reshape(OUT_CH, 9))),
        "dwow": f32(np.asarray(inp["to_out_dw"], np.float32).reshape(OUT_CH, 9)[perm]),
        "dwod": bf(_dw_diag(
            np.asarray(inp["to_kv_dw"], np.float32).reshape(IN_CH, 9),
            np.asarray(inp["to_q_dw"], np.float32).reshape(OUT_CH, 9),
            np.asarray(inp["to_out_dw"], np.float32).reshape(OUT_CH, 9)[perm])),
        "idm": bf(np.eye(P, dtype=np.float32)),
        "rt16": bf(np.kron(_interp_matrix(H1, R), _interp_matrix(W1, R)).T),
        "r64c": bf(_R64_PACKED),
        "selq1": bf(selq1),
        "selq2": bf(selq2),
        "selb": bf(selb),
        "bvt": bf(_rel_bias_small(inp["rel_table"]).transpose(2, 1, 0)
                  .reshape(HEADS * NS, NS)),
        "gkb": bf(np.tile(np.asarray(inp["normk_g"], np.float32).reshape(1, OUT_CH), (P, 1))),
        "bkb": bf(np.tile(np.asarray(inp["normk_b"], np.float32).reshape(1, OUT_CH), (P, 1))),
        "gqb": bf(gq.reshape(OUT_CH, 1)),
        "bqb": bf(np.asarray(inp["normq_b"], np.float32).reshape(OUT_CH, 1)),
    }
    pk = np.zeros((P, 18), np.float32)
    pk[:, 0:4] = np.asarray(inp["norm_l_g"], np.float32).reshape(4, P).T
    pk[:, 4:8] = np.asarray(inp["norm_l_b"], np.float32).reshape(4, P).T
    pk[:, 8:10] = np.asarray(inp["norm_h_g"], np.float32).reshape(2, P).T
    pk[:, 10:12] = np.asarray(inp["norm_h_b"], np.float32).reshape(2, P).T
    pk[:, 12:14] = np.asarray(inp["norm2_g"], np.float32).reshape(2, P).T
    pk[:, 14:16] = np.asarray(inp["norm2_b"], np.float32).reshape(2, P).T
    pk[:, 16:18] = np.asarray(inp["conv_ch_b"], np.float32).reshape(2, P).T
    d["bnpk"] = pk
    return d


# ---------------------------------------------------------------- device build

def _img(ap, w):
    return ap.rearrange("p (h w) -> p h w", w=w)


def _emit_dw_pe_taps(nc, pspool, diag, dslot, xpad, Hs, Ws, pw, name):
    """Taps 1..4 on the TensorEngine: diagonal-weight matmuls accumulated
    in PSUM over shifted padded-image views. Returns the PSUM tiles, one
    per 512-pixel output chunk (tap-outer loop reuses LDWEIGHTS)."""
    nch = (Hs * Ws) // 512
    rpc = 512 // Ws                     # rows per 512-px chunk
    xv = _img(xpad, pw)
    paccs = []
    for w0 in range(0, nch, 4):
        cs = list(range(w0, min(w0 + 4, nch)))
        accs = [pspool.tile([P, 512], FP32, tag="mm512", name=f"{name}{c}")
                for c in cs]
        for j in range(4):
            dy, dx = TAPS[1 + j]
            for a, c in zip(accs, cs):
                nc.tensor.matmul(a[:],
                                 diag[:, dslot + j, :],
                                 xv[:, dy + rpc * c:dy + rpc * c + rpc,
                                    dx:dx + Ws],
                                 start=(j == 0), stop=(j == 3))
        paccs += accs
    return paccs


def _emit_dw_taps(nc, tmps, out, xpad, wvec, Hs, Ws, pw, n_scalar=4,
                  pe=False):
    """Non-PE taps. With pe=False: DVE tensor_scalar tap0 + STT taps
    1..(8-n_scalar), Scalar copies the last n_scalar taps. With pe=True
    (taps 1-4 done by _emit_dw_pe_taps): DVE tap0 (+ STT for taps
    5+n_scalar..8), Scalar taps 5..4+n_scalar. GpSimd is deliberately NOT
    used: its big TT ops saturate SBUF bandwidth and slow concurrent DVE
    streams ~3x (measured)."""
    dst = _img(out, Ws)
    xv = _img(xpad, pw)
    src = lambda i: xv[:, TAPS[i][0]:TAPS[i][0] + Hs, TAPS[i][1]:TAPS[i][1] + Ws]
    nc.vector.tensor_scalar(dst, src(0), wvec[:, 0:1], None, ALU.mult)
    base = 5 if pe else 9 - n_scalar
    dve_taps = (range(5 + n_scalar, 9) if pe else range(1, 9 - n_scalar))
    for i in dve_taps:
        nc.vector.scalar_tensor_tensor(dst, src(i), wvec[:, i:i + 1], dst,
                                       ALU.mult, ALU.add)
    for j in range(n_scalar):
        i = base + j
        nc.scalar.activation(_img(tmps[j][:], Ws), src(i), ACTF.Identity,
                             bias=0.0, scale=wvec[:, i:i + 1])


def _emit_dw_merge(nc, tmps, out, paccs, n_scalar=4):
    """DVE folds the PE-tap PSUM partials first (frees PSUM banks so the
    next wave's matmuls aren't stalled behind the scalar copies), then
    pair-trees the scalar tap copies into out (bf16 2x TT)."""
    for c, a in enumerate(paccs):
        nc.vector.tensor_add(out[:, 512 * c:512 * c + 512],
                             out[:, 512 * c:512 * c + 512], a[:])
    for j in range(n_scalar // 2):
        nc.vector.tensor_add(tmps[2 * j][:], tmps[2 * j][:], tmps[2 * j + 1][:])
    for j in range(n_scalar // 2):
        nc.vector.tensor_add(out, out, tmps[2 * j][:])
    if n_scalar % 2:
        nc.vector.tensor_add(out, out, tmps[n_scalar - 1][:])


def _emit_dw_affine(nc, out, bnS, ew0, ewe, ewc, Hs, Ws):
    """Apply BN affine after a raw depthwise: out = s*out + t*W9 interior,
    with border corrections. All DVE: the main pass is one two-scalar
    tensor_scalar (4x mode), the 8 border fixes are tiny TS adds.
    ew0 [p,1] = t*W9; ewe [p,4] = -t*[r0,r2,c0,c2]; ewc [p,4] = t*corner taps."""
    nc.vector.tensor_scalar(out, out, bnS, ew0, ALU.mult, ALU.add)
    v = _img(out, Ws)
    N = Hs * Ws
    ts = lambda ap, b: nc.vector.tensor_scalar(ap, ap, b, None, ALU.add)
    ts(out[:, 0:Ws], ewe[:, 0:1])                  # top row: -t*r0
    ts(out[:, N - Ws:N], ewe[:, 1:2])              # bottom:  -t*r2
    ts(v[:, :, 0:1], ewe[:, 2:3])                  # left col: -t*c0
    ts(v[:, :, Ws - 1:Ws], ewe[:, 3:4])            # right:    -t*c2
    # corners: add back the doubly-subtracted corner tap
    ts(v[:, 0, 0:1], ewc[:, 0:1])
    ts(v[:, 0, Ws - 1:Ws], ewc[:, 1:2])
    ts(v[:, Hs - 1, 0:1], ewc[:, 2:3])
    ts(v[:, Hs - 1, Ws - 1:Ws], ewc[:, 3:4])


def _pad_memsets(nc, xpad, Hs, Ws, pw):
    """Zero only the pad cells of a [p, (Hs+2)*pw] image buffer (GpSimd)."""
    # top row + leading left-pad cell of first interior row
    nc.gpsimd.memset(xpad[:, 0:pw + 1], 0.0)
    # per interior row: trailing right-pad + next row's left-pad (2 cells,
    # adjacent because pw == Ws + 2)
    mid = xpad[:, pw + Ws + 1:pw + Ws + 1 + (Hs - 1) * pw] \
        .rearrange("p (h c) -> p h c", c=pw)[:, :, 0:2]
    nc.gpsimd.memset(mid, 0.0)
    # bottom pad row + trailing right-pad cell of last interior row
    nc.gpsimd.memset(xpad[:, (Hs + 1) * pw - 1:(Hs + 2) * pw], 0.0)


def _emit(nc, tc, dram, out_d):
    import contextlib
    ctx = contextlib.ExitStack()
    pool = lambda name, bufs, space="SBUF": ctx.enter_context(
        tc.tile_pool(name=name, bufs=bufs, space=space))

    consts = pool("consts", 1)
    work = pool("work", 1)       # unique-tag persistents (small)
    big32 = pool("big32", 1)     # X2 early / OSB late (32KB fp32 class)
    bigA = pool("bigA", 1)       # XP2 early / OPAD late (17.4KB bf16)
    bigB = pool("bigB", 1)       # DWQraw early / DWO late (16KB bf16)
    bigC = pool("bigC", 1)       # X1 fp32 early / Q + RELU later
    big16 = pool("big16", 1)     # remaining 16KB-class persists
    tmpp = pool("tmpp", 1)       # tap tmp buffers [P, N2] bf16 (4 tags)
    tr4 = pool("tr4", 3)         # transient ~4KB chunks
    ps = pool("ps", 4, "PSUM")
    pss = pool("pss", 2, "PSUM")
    dpool = pool("dramp", 1, "DRAM")

    dma = nc.sync.dma_start

    # ---------------- inputs / constants
    X1 = bigC.tile([P, 4, N1], FP32, tag="bigC")
    x1src = dram["x1"].ap().rearrange("(t p) n -> p t n", p=P)
    dma(X1[:, 0:2, :], x1src[:, 0:2, :])
    dma(X1[:, 2:4, :], x1src[:, 2:4, :])

    def load_c(name, shape, dt=FP32):
        t = consts.tile(shape, dt, tag=name)
        src = dram[name].ap()
        if len(shape) == 3:
            src = src.rearrange("(t p) n -> p t n", p=P)
        dma(t[:], src)
        return t

    # order matters: small tiles that gate early work load first
    DW1W = load_c("dw1w", [P, 4, 18])
    DWQW = load_c("dwqw", [P, 2, 18])
    DWOW = load_c("dwow", [P, 2, 9])
    BNPK = load_c("bnpk", [P, 18])
    X2 = big32.tile([P, 2, N2], FP32, tag="big32")
    dma(X2[:], dram["x2"].ap().rearrange("(t p) n -> p t n", p=P))
    WCH = load_c("wch", [P, 4, OUT_CH])
    WKV = load_c("wkv", [P, 4, 2 * OUT_CH], BF16)
    RT16 = load_c("rt16", [P, 8, NS], BF16)
    WQ = load_c("wq", [P, 2, OUT_CH], BF16)
    SELQ1 = load_c("selq1", [P, 2, HEADS], BF16)
    SELQ2 = load_c("selq2", [P, 2, HEADS], BF16)
    GKB = load_c("gkb", [P, OUT_CH], BF16)
    BKB = load_c("bkb", [P, OUT_CH], BF16)
    GQB = load_c("gqb", [P, 2, 1], BF16)
    BQB = load_c("bqb", [P, 2, 1], BF16)
    BVT = load_c("bvt", [P, 2 * HEADS, NS], BF16)
    SELB = load_c("selb", [P, 16, OUT_CH], BF16)
    WOUT = load_c("wout", [P, 2, OUT_CH], BF16)
    WMLP = load_c("wmlp", [P, 2, OUT_CH], BF16)

    IDM = load_c("idm", [P, P], BF16)
    # diag-tap tables for the PE-side taps of DWQ (slots 0-7) and DWO (8-15)
    DIAG = tmpp.tile([P, 16, P], BF16, tag="tmp4", name="dwdiag")
    dma(DIAG[:], dram["dwod"].ap().rearrange("(t p) n -> p t n", p=P)[:, 16:32, :])

    # ---------------- padded raw images + BN stats (Scalar engine, fused)
    XP1 = [work.tile([P, PAD1], BF16, tag=f"XP1_{t}", name=f"XP1_{t}")
           for t in range(4)]
    XP2 = bigA.tile([P, 2, PAD2], BF16, tag="bigA")
    for t in range(4):
        _pad_memsets(nc, XP1[t][:], H1, W1, PW1)
    for t in range(2):
        _pad_memsets(nc, XP2[:, t, :], H2, W2, PW2)

    # layout: 0..7 x1 (S,S2)x4; 8,9 x2 S; 10..13 x2-t0 S2 chunks; 14..17 t1
    ccin = work.tile([P, 18], FP32, tag="ccin")
    trash = work.tile([P, N1], BF16, tag="trash")
    # x1 stats on DVE (idle at the head) so the scalar queue reaches the
    # x2 stats -- which gate the collective -- sooner
    st1 = work.tile([P, 4, 12], FP32, tag="st1")
    ag1 = work.tile([P, 4, 2], FP32, tag="ag1")
    for t in range(4):
        for c in range(2):
            nc.vector.bn_stats(st1[:, t, 6 * c:6 * c + 6], X1[:, t, bass.ts(c, 512)])
        nc.vector.bn_aggr(ag1[:, t, :],
                          st1[:, t, :].rearrange("p (c s) -> p c s", s=6))
        m, v = ag1[:, t, 0:1], ag1[:, t, 1:2]
        S, S2 = ccin[:, 2 * t:2 * t + 1], ccin[:, 2 * t + 1:2 * t + 2]
        nc.vector.tensor_scalar(S, m, float(N1), None, ALU.mult)
        nc.vector.tensor_mul(S2, m, m)
        nc.vector.tensor_add(S2, S2, v)
        nc.vector.tensor_scalar(S2, S2, float(N1), None, ALU.mult)
    for t in range(4):
        nc.scalar.activation(_img(XP1[t][:], PW1)[:, 1:1 + H1, 1:1 + W1],
                             _img(X1[:, t, :], W1), ACTF.Identity, bias=0.0)
    for t in range(2):
        nc.scalar.activation(_img(XP2[:, t, :], PW2)[:, 1:1 + H2, 1:1 + W2],
                             _img(X2[:, t, :], W2), ACTF.Identity, bias=0.0,
                             accum_out=ccin[:, 8 + t:9 + t])
        for c in range(4):
            nc.scalar.activation(trash[:], X2[:, t, bass.ts(c, N1)], ACTF.Square,
                                 accum_out=ccin[:, 10 + 4 * t + c:11 + 4 * t + c])

    # collective input DMA dispatched from the (idle) GpSimd queue so it
    # doesn't sit behind the const loads on the Sync queue
    cc1i = dpool.tile([P, 18], FP32, tag="cc1i")
    cc1o = dpool.tile([P, 18], FP32, tag="cc1o")
    nc.gpsimd.dma_start(cc1i[:], ccin[:])
    nc.gpsimd.collective_compute("AllReduce", ALU.add,
                                 replica_groups=[list(range(NCORES))],
                                 ins=[cc1i.opt()], outs=[cc1o.opt()])
    ccout = work.tile([P, 18], FP32, tag="ccout")

    # ---------------- conv_ch transposed (for the residue, consumed late)
    X1CT = work.tile([P, 8, OUT_CH], BF16, tag="X1CT")
    for m in range(8):
        acc = ps.tile([P, 512], FP32, tag="mm512")
        for kk in range(4):
            nc.tensor.matmul(acc[:, 0:OUT_CH], X1[:, kk, bass.ts(m, P)],
                             WCH[:, kk, :], start=(kk == 0), stop=(kk == 3))
        nc.scalar.copy(X1CT[:, m, :], acc[:, 0:OUT_CH])

    # ---------------- raw depthwise (runs during the AllReduce flight)
    # taps 1-4 of every depthwise tile run on the TensorEngine (idle in this
    # window) as diagonal matmuls; tap 0 is a DVE tensor_scalar; taps 5-8
    # split between DVE STT and Scalar copies depending on who is free
    DW1 = work.tile([P, 4, N1], BF16, tag="DW1")
    tmps1 = [tmpp.tile([P, N2], BF16, tag=f"tmp{j}", name=f"tmp{j}")
             for j in range(5)]
    for t in range(4):
        ns = 0 if t < 2 else 4
        sub = [SubTile(tt, N1) for tt in tmps1]
        _emit_dw_taps(nc, sub, DW1[:, t, :], XP1[t][:], DW1W[:, t, :],
                      H1, W1, PW1, n_scalar=ns)
        _emit_dw_merge(nc, sub, DW1[:, t, :], [], n_scalar=ns)
    DWQ = bigB.tile([P, 2, N2], BF16, tag="bigB")

    # ---------------- BN scale/shift sandwich (DVE reaches this right as
    # the collective returns); ccout fetched from the DVE queue
    nc.scalar.dma_start(ccout[:], cc1o[:])
    bnS = work.tile([P, 6], FP32, tag="bnS")
    bnT = work.tile([P, 6], FP32, tag="bnT")
    mean6 = work.tile([P, 6], FP32, tag="mean6")
    var6 = work.tile([P, 6], FP32, tag="var6")
    s2x2 = work.tile([P, 2], FP32, tag="s2x2")
    for t in range(2):
        nc.vector.tensor_reduce(s2x2[:, t:t + 1],
                                ccout[:, 10 + 4 * t:14 + 4 * t],
                                mybir.AxisListType.X, ALU.add, opt_input=False)
    for t in range(6):
        n = float(B * (N1 if t < 4 else N2))
        if t < 4:
            S, S2 = ccout[:, 2 * t:2 * t + 1], ccout[:, 2 * t + 1:2 * t + 2]
        else:
            S, S2 = ccout[:, 8 + (t - 4):9 + (t - 4)], s2x2[:, t - 4:t - 3]
        m, v = mean6[:, t:t + 1], var6[:, t:t + 1]
        nc.vector.tensor_scalar(m, S, 1.0 / n, None, ALU.mult)
        nc.vector.scalar_tensor_tensor(v, m, -1.0, m, ALU.mult, ALU.mult)
        nc.vector.scalar_tensor_tensor(v, S2, 1.0 / n, v, ALU.mult, ALU.add)
        nc.vector.tensor_scalar(v, v, EPS_BN, None, ALU.add)
    nc.vector.reciprocal(var6[:], var6[:])
    nc.scalar.activation(bnS[:], var6[:], ACTF.Sqrt)
    nc.vector.tensor_mul(bnS[:, 0:4], bnS[:, 0:4], BNPK[:, 0:4])
    nc.vector.tensor_mul(bnS[:, 4:6], bnS[:, 4:6], BNPK[:, 8:10])
    nc.vector.tensor_mul(mean6[:], mean6[:], bnS[:])
    nc.vector.tensor_sub(bnT[:, 0:4], BNPK[:, 4:8], mean6[:, 0:4])
    nc.vector.tensor_sub(bnT[:, 4:6], BNPK[:, 10:12], mean6[:, 4:6])

    # edge scalars: ew0 = t*W9; ewe = -t*[r0,r2,c0,c2]; ewc = t*corners
    negT = work.tile([P, 6], FP32, tag="negT")
    nc.vector.tensor_scalar(negT[:], bnT[:], -1.0, None, ALU.mult)
    EW01 = work.tile([P, 4, 1], FP32, tag="EW01")
    EWE1 = work.tile([P, 4, 4], FP32, tag="EWE1")
    EWC1 = work.tile([P, 4, 4], FP32, tag="EWC1")
    EW0Q = work.tile([P, 2, 1], FP32, tag="EW0Q")
    EWEQ = work.tile([P, 2, 4], FP32, tag="EWEQ")
    EWCQ = work.tile([P, 2, 4], FP32, tag="EWCQ")
    for t in range(4):
        nc.vector.tensor_scalar(EW01[:, t, :], DW1W[:, t, 9:10],
                                bnT[:, t:t + 1], None, ALU.mult)
        nc.vector.tensor_scalar(EWE1[:, t, :], DW1W[:, t, 10:14],
                                negT[:, t:t + 1], None, ALU.mult)
        nc.vector.tensor_scalar(EWC1[:, t, :], DW1W[:, t, 14:18],
                                bnT[:, t:t + 1], None, ALU.mult)
    for t in range(2):
        nc.vector.tensor_scalar(EW0Q[:, t, :], DWQW[:, t, 9:10],
                                bnT[:, 4 + t:5 + t], None, ALU.mult)
        nc.vector.tensor_scalar(EWEQ[:, t, :], DWQW[:, t, 10:14],
                                negT[:, 4 + t:5 + t], None, ALU.mult)
        nc.vector.tensor_scalar(EWCQ[:, t, :], DWQW[:, t, 14:18],
                                bnT[:, 4 + t:5 + t], None, ALU.mult)

    # first DWQ tile (scalar tap copies already queued ahead of the affines)
    paccq0 = _emit_dw_pe_taps(nc, ps, DIAG, 0, XP2[:, 0, :],
                              H2, W2, PW2, "dqp0_")
    _emit_dw_taps(nc, tmps1, DWQ[:, 0, :], XP2[:, 0, :], DWQW[:, 0, :],
                  H2, W2, PW2, n_scalar=3, pe=True)
    _emit_dw_merge(nc, tmps1, DWQ[:, 0, :], paccq0, n_scalar=3)

    # apply BN affine to DW1 (Scalar engine only), then kv pointwise can start
    for t in range(4):
        _emit_dw_affine(nc, DW1[:, t, :], bnS[:, t:t + 1], EW01[:, t, :],
                        EWE1[:, t, :], EWC1[:, t, :], H1, W1)

    # ---------------- kv pointwise (pixel-major out)
    KVT = big16.tile([P, 8, 2 * OUT_CH], BF16, tag="big16")
    for m in range(8):
        acc = ps.tile([P, 512], FP32, tag="mm512")
        for kk in range(4):
            nc.tensor.matmul(acc[:], DW1[:, kk, bass.ts(m, P)], WKV[:, kk, :],
                             start=(kk == 0), stop=(kk == 3))
        nc.vector.tensor_copy(KVT[:, m, :], acc[:])

    # second DWQ tile + both affines
    paccq1 = _emit_dw_pe_taps(nc, ps, DIAG, 4, XP2[:, 1, :],
                              H2, W2, PW2, "dqp1_")
    _emit_dw_taps(nc, tmps1, DWQ[:, 1, :], XP2[:, 1, :], DWQW[:, 1, :],
                  H2, W2, PW2, n_scalar=3, pe=True)
    _emit_dw_merge(nc, tmps1, DWQ[:, 1, :], paccq1, n_scalar=3)
    for t in range(2):
        _emit_dw_affine(nc, DWQ[:, t, :], bnS[:, 4 + t:5 + t], EW0Q[:, t, :],
                        EWEQ[:, t, :], EWCQ[:, t, :], H2, W2)

    # resize 32->16: kvsT = RT16^T @ KVT  [256 smallpix, 512]
    KVS = []
    for mm in range(2):
        acc = pss.tile([P, 512], FP32, tag="psmall")
        for kk in range(8):
            nc.tensor.matmul(acc[:], RT16[:, kk, bass.ts(mm, P)], KVT[:, kk, :],
                             start=(kk == 0), stop=(kk == 7))
        KVS.append(acc)

    # LN-k + evac k' ; v' plain evac (bf16)
    KP = work.tile([P, 2, OUT_CH], BF16, tag="KP")
    VP = work.tile([P, 2, OUT_CH], BF16, tag="VP")
    ksq = work.tile([P, OUT_CH], FP32, tag="ksq")
    ksum = work.tile([P, HEADS], FP32, tag="ksum")
    km = work.tile([P, HEADS], FP32, tag="km")
    krs = work.tile([P, HEADS], FP32, tag="krs")
    kfp = work.tile([P, OUT_CH], FP32, tag="kfp")
    for mm in range(2):
        k_ap = KVS[mm][:, 0:OUT_CH].rearrange("p (h d) -> p h d", d=DIM_HEAD)
        nc.vector.tensor_reduce(ksum[:], k_ap, mybir.AxisListType.X, ALU.add,
                                opt_input=False)
        nc.scalar.activation(ksq[:], KVS[mm][:, 0:OUT_CH], ACTF.Square)
        nc.vector.tensor_reduce(krs[:], ksq[:].rearrange("p (h d) -> p h d",
                                                         d=DIM_HEAD),
                                mybir.AxisListType.X, ALU.add, opt_input=False)
        nc.vector.scalar_tensor_tensor(km[:], ksum[:], -1.0 / DIM_HEAD, ksum[:],
                                       ALU.mult, ALU.mult)
        nc.vector.tensor_add(krs[:], krs[:], km[:])
        nc.vector.tensor_scalar(krs[:], krs[:], DIM_HEAD * EPS_LN, None, ALU.add)
        nc.vector.reciprocal(krs[:], krs[:])
        nc.scalar.activation(krs[:], krs[:], ACTF.Sqrt, scale=float(DIM_HEAD))
        nc.vector.tensor_scalar(km[:], ksum[:], 1.0 / DIM_HEAD, None, ALU.mult)
        kb = km[:].unsqueeze(2).broadcast_to([P, HEADS, DIM_HEAD])
        rb = krs[:].unsqueeze(2).broadcast_to([P, HEADS, DIM_HEAD])
        t1 = kfp[:].rearrange("p (h d) -> p h d", d=DIM_HEAD)
        nc.vector.tensor_sub(t1, k_ap, kb)
        nc.vector.tensor_mul(t1, t1, rb)
        nc.vector.tensor_mul(kfp[:], kfp[:], GKB[:])
        nc.vector.tensor_add(KP[:, mm, :], kfp[:], BKB[:])
        nc.vector.tensor_copy(VP[:, mm, :], KVS[mm][:, OUT_CH:2 * OUT_CH])

    # A = K'^T V' / 32 : diagonal head blocks packed block-diagonal
    BD = work.tile([P, 2, P], BF16, tag="BD")
    nc.gpsimd.memset(BD[:], 0.0)
    for mo in range(2):
        acc = pss.tile([P, 512], FP32, tag="psmall")
        for kk in range(2):
            nc.tensor.matmul(acc[:, 0:OUT_CH], KP[:, kk, bass.ts(mo, P)],
                             VP[:, kk, :], start=(kk == 0), stop=(kk == 1))
        for hh in range(4):
            h = mo * 4 + hh
            nc.scalar.activation(BD[bass.ds(32 * hh, 32), mo, bass.ds(32 * hh, 32)],
                                 acc[bass.ds(32 * hh, 32), bass.ds(32 * h, 32)],
                                 ACTF.Copy, scale=1.0 / DIM_HEAD)

    # Bb = BD @ b_q, Gg = BD @ g (per-channel consts for deferred LN-q)
    BbGg = work.tile([P, 2, 2], FP32, tag="BbGg")   # [:, pk, 0]=Bb, 1=-Gg
    for pk in range(2):
        acc = pss.tile([P, 512], FP32, tag="psmall")
        nc.tensor.matmul(acc[:, 0:1], BD[:, pk, :], BQB[:, pk, :],
                         start=True, stop=True)
        nc.tensor.matmul(acc[:, 1:2], BD[:, pk, :], GQB[:, pk, :],
                         start=True, stop=True)
        nc.scalar.copy(BbGg[:, pk, 0:1], acc[:, 0:1])
        nc.scalar.activation(BbGg[:, pk, 1:2], acc[:, 1:2], ACTF.Identity,
                             bias=0.0, scale=-1.0)

    # BV[(h,d'), is] = (v'^T bias_small_h)/32 via full-M matmul + row extract
    BVC = work.tile([P, 2, NS], BF16, tag="BVC")
    for h in range(HEADS):
        mo, hh = h // 4, h % 4
        acc = pss.tile([P, 512], FP32, tag="psmall")
        for kk in range(2):
            nc.tensor.matmul(acc[:, 0:NS], VP[:, kk, bass.ts(mo, P)],
                             BVT[:, 2 * h + kk, :], start=(kk == 0), stop=(kk == 1))
        nc.scalar.activation(BVC[bass.ds(32 * hh, 32), mo, :],
                             acc[bass.ds(32 * hh, 32), 0:NS],
                             ACTF.Copy, scale=1.0 / DIM_HEAD)
    # expand along x: BVX[c, ys*64 + x] = BVC[c, ys*16 + x//4]; then += Bb
    BVX = work.tile([P, 2, R * W2], BF16, tag="BVX")
    for mo in range(2):
        nc.vector.tensor_copy(
            BVX[:, mo, :].rearrange("p (ys xs xr) -> p ys xs xr", xs=R, xr=4),
            BVC[:, mo, :].rearrange("p (ys xs) -> p ys xs", xs=R)
            .unsqueeze(3).broadcast_to([P, R, R, 4]))
        nc.vector.tensor_scalar(BVX[:, mo, :], BVX[:, mo, :],
                                BbGg[:, mo, 0:1], None, ALU.add)

    # ---------------- q pointwise (g-folded) + LN-q stats
    Q = bigC.tile([P, 2, N2], BF16, tag="bigC")     # reuses X1 slot
    QSP = work.tile([P, 2, NS], FP32, tag="QSP")   # [(h*16+blk), (qs|q2s), 256]
    for nn in range(8):
        q2c = tr4.tile([P, 2, 512], BF16, tag="tr4")
        for mm in range(2):
            acc = ps.tile([P, 512], FP32, tag="mm512")
            for kk in range(2):
                nc.tensor.matmul(acc[:], WQ[:, kk, bass.ts(mm, P)],
                                 DWQ[:, kk, bass.ts(nn, 512)],
                                 start=(kk == 0), stop=(kk == 1))
            nc.scalar.copy(Q[:, mm, bass.ts(nn, 512)], acc[:])
            nc.vector.tensor_mul(q2c[:, mm, :], Q[:, mm, bass.ts(nn, 512)],
                                 Q[:, mm, bass.ts(nn, 512)])
        for s in range(2):
            sacc = pss.tile([P, 512], FP32, tag="psmall")
            SEL = SELQ1 if s == 0 else SELQ2
            for mm in range(2):
                rhs = Q[:, mm, bass.ts(nn, 512)] if s == 0 else q2c[:, mm, :]
                nc.tensor.matmul(sacc[0:HEADS, :], SEL[:, mm, :], rhs,
                                 start=(mm == 0), stop=(mm == 1))
            qsc = tr4.tile([HEADS, 512], FP32, tag="tr4")
            nc.vector.tensor_copy(qsc[:], sacc[0:HEADS, :])
            # relayout rows: row 16nn + 2h + b  <->  (blk = 2nn+b, h)
            dma(QSP[bass.ds(16 * nn, 16), s, :],
                qsc[:].rearrange("h (b f) -> h b f", f=NS))

    # rs | mrs  (bf16, packed for the broadcast matmul)
    RSP = work.tile([P, 2, NS], BF16, tag="RSP")
    numt = work.tile([P, NS], FP32, tag="numt")
    qsv, q2v = QSP[:, 0, :], QSP[:, 1, :]
    nc.vector.scalar_tensor_tensor(numt[:], qsv, -1.0 / DIM_HEAD, qsv, ALU.mult, ALU.mult)
    nc.vector.tensor_add(numt[:], numt[:], q2v)
    nc.vector.tensor_scalar(numt[:], numt[:], DIM_HEAD * EPS_LN, None, ALU.add)
    nc.vector.reciprocal(numt[:], numt[:])
    nc.scalar.activation(RSP[:, 0, :], numt[:], ACTF.Sqrt, scale=float(DIM_HEAD))
    nc.vector.scalar_tensor_tensor(RSP[:, 1, :], qsv, 1.0 / DIM_HEAD, RSP[:, 0, :],
                                   ALU.mult, ALU.mult)

    # ---------------- per-256-pixel block: broadcast stats, QA matmul on
    # g-folded Q, deferred LN affine on the output, add bias map, write OPAD
    OPAD = bigA.tile([P, 2, PAD2], BF16, tag="bigA")   # reuses XP2 slot
    for t in range(2):
        _pad_memsets(nc, OPAD[:, t, :], H2, W2, PW2)
    rsp_flat = RSP[:].rearrange("p s f -> p (s f)")
    for blk in range(16):
        rsb = tr4.tile([P, 2, 2, NS], BF16, tag="tr4")
        for mm in range(2):
            bacc = pss.tile([P, 512], FP32, tag="psmall")
            nc.tensor.matmul(bacc[:], SELB[:, blk, bass.ts(mm, P)], rsp_flat,
                             start=True, stop=True)
            nc.scalar.copy(rsb[:, mm, :, :],
                           bacc[:].rearrange("p (s f) -> p s f", f=NS))
        for pk in range(2):
            acc = ps.tile([P, 512], FP32, tag="mm512")
            nc.tensor.matmul(acc[:, 0:NS], BD[:, pk, :],
                             Q[:, pk, bass.ds(blk * NS, NS)],
                             start=True, stop=True)
            tmpo = tr4.tile([P, NS], BF16, tag="tr4b")
            nc.vector.tensor_mul(tmpo[:], acc[:, 0:NS], rsb[:, pk, 0, :])
            nc.vector.scalar_tensor_tensor(tmpo[:], rsb[:, pk, 1, :],
                                           BbGg[:, pk, 1:2], tmpo[:],
                                           ALU.mult, ALU.add)
            # rows 4*blk .. 4*blk+4 of the 64x64 image; ys = blk
            dst = _img(OPAD[:, pk, :], PW2)[:, 1 + 4 * blk:5 + 4 * blk, 1:1 + W2]
            bv = BVX[:, pk, bass.ds(blk * W2, W2)].unsqueeze(1) \
                .broadcast_to([P, 4, W2])
            nc.gpsimd.tensor_add(dst,
                                 tmpo[:].rearrange("p (yr w) -> p yr w", w=W2),
                                 bv)

    # ---------------- to_out depthwise + pointwise + residue in PSUM
    # taps 1-4 run on the (otherwise idle) TensorEngine as diagonal-weight
    # matmuls PSUM-accumulated over shifted OPAD views; tap 0 is a DVE
    # tensor_scalar; taps 5-8 are scalar-engine copies merged on DVE.
    DWO = bigB.tile([P, 2, N2], BF16, tag="bigB")   # reuses DWQ slot
    for t in range(2):
        pacco = _emit_dw_pe_taps(nc, ps, DIAG, 8 + 4 * t, OPAD[:, t, :],
                                 H2, W2, PW2, f"dop{t}_")
        _emit_dw_taps(nc, tmps1, DWO[:, t, :], OPAD[:, t, :], DWOW[:, t, :],
                      H2, W2, PW2, n_scalar=4, pe=True)
        _emit_dw_merge(nc, tmps1, DWO[:, t, :], pacco, n_scalar=4)
    # residue-resize table loads into the freed tap-tmp slabs (3 chunks of 8)
    r64src = dram["r64c"].ap().rearrange("(t p) n -> p t n", p=P)
    R64T = []
    for c in range(3):
        lo, hi = 8 * c, min(8 * c + 8, _N_R64_SLOTS)
        rc = tmpp.tile([P, hi - lo, 512], BF16, tag=f"tmp{c}", name=f"r64t{c}")
        dma(rc[:], r64src[:, lo:hi, :])
        R64T.append(rc)
    OSB = big32.tile([P, 2, N2], BF16, tag="big32")   # reuses X2 slot
    # OSB evac (+conv_ch bias) and BN2 stats on DVE -- it is idle during
    # the WOUT matmul phase while the scalar engine was the pacer before
    st2 = work.tile([P, 2, 48], FP32, tag="st2")
    ag2 = work.tile([P, 2, 2], FP32, tag="ag2")
    slot = 0
    for nn in range(8):
        used = _R64_KTILES[nn]
        for mm in range(2):
            acc = ps.tile([P, 512], FP32, tag="mm512")
            for kk in range(2):
                nc.tensor.matmul(acc[:], WOUT[:, kk, bass.ts(mm, P)],
                                 DWO[:, kk, bass.ts(nn, 512)],
                                 start=(kk == 0), stop=False)
            for i, kk in enumerate(used):
                s = slot + i
                nc.tensor.matmul(acc[:], X1CT[:, kk, bass.ts(mm, P)],
                                 R64T[s // 8][:, s % 8, :],
                                 start=False, stop=(i == len(used) - 1))
            nc.vector.tensor_scalar(OSB[:, mm, bass.ts(nn, 512)], acc[:],
                                    BNPK[:, 16 + mm:17 + mm], None, ALU.add)
            nc.vector.bn_stats(st2[:, mm, 6 * nn:6 * nn + 6],
                               OSB[:, mm, bass.ts(nn, 512)])
        slot += len(used)

    # ---------------- BN2 (norm2) stats reduce + AllReduce
    cc2s = work.tile([P, 4], FP32, tag="cc2s")
    for t in range(2):
        nc.vector.bn_aggr(ag2[:, t, :],
                          st2[:, t, :].rearrange("p (c s) -> p c s", s=6))
        m, v = ag2[:, t, 0:1], ag2[:, t, 1:2]
        S, S2 = cc2s[:, 2 * t:2 * t + 1], cc2s[:, 2 * t + 1:2 * t + 2]
        nc.vector.tensor_scalar(S, m, float(N2), None, ALU.mult)
        nc.vector.tensor_mul(S2, m, m)
        nc.vector.tensor_add(S2, S2, v)
        nc.vector.tensor_scalar(S2, S2, float(N2), None, ALU.mult)
    cc2i = dpool.tile([P, 4], FP32, tag="cc2i")
    cc2o = dpool.tile([P, 4], FP32, tag="cc2o")
    nc.gpsimd.dma_start(cc2i[:], cc2s[:])
    nc.gpsimd.collective_compute("AllReduce", ALU.add,
                                 replica_groups=[list(range(NCORES))],
                                 ins=[cc2i.opt()], outs=[cc2o.opt()])
    cc2r = work.tile([P, 4], FP32, tag="cc2r")
    nc.scalar.dma_start(cc2r[:], cc2o[:])
    bn3S = work.tile([P, 2], FP32, tag="bn3S")
    bn3T = work.tile([P, 2], FP32, tag="bn3T")
    m3 = work.tile([P, 2], FP32, tag="m3")
    v3 = work.tile([P, 2], FP32, tag="v3")
    nB = float(B * N2)
    for t in range(2):
        S, S2 = cc2r[:, 2 * t:2 * t + 1], cc2r[:, 2 * t + 1:2 * t + 2]
        nc.vector.tensor_scalar(m3[:, t:t + 1], S, 1.0 / nB, None, ALU.mult)
        nc.vector.scalar_tensor_tensor(v3[:, t:t + 1], m3[:, t:t + 1], -1.0,
                                       m3[:, t:t + 1], ALU.mult, ALU.mult)
        nc.vector.scalar_tensor_tensor(v3[:, t:t + 1], S2, 1.0 / nB,
                                       v3[:, t:t + 1], ALU.mult, ALU.add)
        nc.vector.tensor_scalar(v3[:, t:t + 1], v3[:, t:t + 1], EPS_BN, None, ALU.add)
    nc.vector.reciprocal(v3[:], v3[:])
    nc.scalar.activation(bn3S[:], v3[:], ACTF.Sqrt)
    nc.vector.tensor_mul(bn3S[:], bn3S[:], BNPK[:, 12:14])
    nc.vector.tensor_mul(m3[:], m3[:], bn3S[:])
    nc.vector.tensor_sub(bn3T[:], BNPK[:, 14:16], m3[:])

    # ---------------- relu(bn) + mlp + final residual -> out
    # relu is chunked into the mlp loop so the first matmul starts ~6us
    # earlier after the BN2 collective returns
    RELU = bigC.tile([P, 2, N2], BF16, tag="bigC")   # reuses Q slot
    out_ap = out_d.ap().rearrange("(t p) n -> p t n", p=P)
    for nn in range(8):
        for t in range(2):
            nc.scalar.activation(RELU[:, t, bass.ts(nn, 512)],
                                 OSB[:, t, bass.ts(nn, 512)], ACTF.Relu,
                                 bias=bn3T[:, t:t + 1], scale=bn3S[:, t:t + 1])
        for mm in range(2):
            acc = ps.tile([P, 512], FP32, tag="mm512")
            for kk in range(2):
                nc.tensor.matmul(acc[:], WMLP[:, kk, bass.ts(mm, P)],
                                 RELU[:, kk, bass.ts(nn, 512)],
                                 start=(kk == 0), stop=(kk == 1))
            fin = tr4.tile([P, 512], FP32, tag="tr4")
            nc.vector.tensor_add(fin[:], acc[:], OSB[:, mm, bass.ts(nn, 512)])
            dma(out_ap[:, mm, bass.ts(nn, 512)], fin[:])

    ctx.close()


class SubTile:
    """View adapter: presents the first n columns of a tile as a tile."""
    def __init__(self, t, n):
        self._t = t
        self._n = n

    def __getitem__(self, key):
        return self._t[:, 0:self._n]


def _build_program():
    nc = bacc.Bacc("TRN2", target_bir_lowering=False, debug=False,
                   num_devices=NCORES)
    dram = {}

    def din(name, shape, dt=FP32):
        dram[name] = nc.dram_tensor(name, list(shape), dt, kind="ExternalInput")

    din("x1", (IN_CH, N1)); din("x2", (OUT_CH, N2))
    din("wch", (IN_CH, OUT_CH)); din("wkv", (IN_CH, 2 * OUT_CH), BF16)
    din("wq", (OUT_CH, OUT_CH), BF16); din("wout", (OUT_CH, OUT_CH), BF16)
    din("wmlp", (OUT_CH, OUT_CH), BF16)
    din("dw1w", (IN_CH, 18)); din("dwqw", (OUT_CH, 18)); din("dwow", (OUT_CH, 9))
    din("dwod", (32 * P, P), BF16)
    din("idm", (P, P), BF16)
    din("rt16", (N1, NS), BF16); din("r64c", (_N_R64_SLOTS * P, 512), BF16)
    din("selq1", (OUT_CH, HEADS), BF16); din("selq2", (OUT_CH, HEADS), BF16)
    din("selb", (16 * P, OUT_CH), BF16)
    din("bvt", (HEADS * NS, NS), BF16)
    din("gkb", (P, OUT_CH), BF16); din("bkb", (P, OUT_CH), BF16)
    din("gqb", (OUT_CH, 1), BF16); din("bqb", (OUT_CH, 1), BF16)
    din("bnpk", (P, 18))
    out_d = nc.dram_tensor("out", [OUT_CH, N2], FP32, kind="ExternalOutput")

    with tile.TileContext(nc) as tc:
        _emit(nc, tc, dram, out_d)
    nc.compile()
    return nc


# ------------------------------------------------------------------- run layer

_CACHE = {}


def _get_program():
    if "nc" not in _CACHE:
        _CACHE["nc"] = _build_program()
    return _CACHE["nc"]


def kernel(**inputs):
    nc = _get_program()
    shared = _host_prep(inputs)
    x1 = np.ascontiguousarray(np.asarray(inputs["x1"], np.float32).reshape(B, IN_CH, N1))
    x2 = np.ascontiguousarray(np.asarray(inputs["x2"], np.float32).reshape(B, OUT_CH, N2))
    in_maps = [dict(shared, x1=x1[b], x2=x2[b]) for b in range(B)]
    res = run_bass_kernel_spmd(nc, in_maps, core_ids=list(range(NCORES)))
    out = np.stack([np.asarray(res.results[b]["out"], np.float32)
                    .reshape(OUT_CH, H2, W2) for b in range(B)])
    return out


# revision 69
# speedup vs baseline: 1.2071x; 1.2071x over previous
"""Trainium2 Bass kernel for nn_BasicTransDecoderBlock (dense_transformer).

Strategy: data-parallel over batch B=8 across 8 NeuronCores (1 sample/core).
BatchNorm batch statistics are synchronized with two small AllReduces.
The attention is softmax-free and reassociated:
    O = Q' (K'^T V')/d  +  (bias @ V')/d
which collapses the dominant QK^T/AV FLOPs into tiny per-head d x d matmuls.

v2 optimizations over the first working version:
 - Depthwise convs run on RAW (pre-BN) zero-padded inputs so they can
   execute during the BN-stats AllReduce flight; the BN affine commutes:
   DW(s*x+t) = s*DWraw(x) + t*kappa, applied afterwards on the Scalar
   engine with tiny per-edge border corrections (kappa deviates from the
   full 3x3 weight sum only at image borders).
 - BN statistics are computed with the Scalar engine's accum_out while
   building the padded bf16 images (one fused pass), not with DVE bn_stats.
 - Depthwise taps are split across DVE (scalar_tensor_tensor), Scalar
   (scaled shifted copies) and GpSimd (pair merges) -- STT is hard-capped
   at 1x on DVE so parallel engines beat more DVE work.
 - LayerNorm-q is algebraically deferred: normq_g folds into the Q
   pointwise weights on host, the per-(head,pixel) rs/mrs scalars apply
   AFTER the block-diagonal attention matmul, and BD@b_q pre-adds into
   the bias map.
 - Pad-region-only memsets on GpSimd; PSUM evacuations on Scalar; the
   upsampled residue accumulates directly in the to_out PSUM tile.

Self-contained: hardcodes all shapes; imports only the concourse runtime
shipped in the container.
"""
import sys
import numpy as np
import ml_dtypes

for _p in ("/opt/trn_rl_repo", "/root/.axon_site/_ro/trn_rl_repo"):
    if _p not in sys.path:
        sys.path.insert(0, _p)

import concourse.bass as bass
import concourse.bacc as bacc
import concourse.tile as tile
from concourse import mybir
from concourse.bass_utils import run_bass_kernel_spmd

FP32 = mybir.dt.float32
BF16 = mybir.dt.bfloat16
ALU = mybir.AluOpType
ACTF = mybir.ActivationFunctionType

B, IN_CH, OUT_CH, HEADS, DIM_HEAD, R = 8, 512, 256, 8, 32, 16
H1, W1, H2, W2 = 32, 32, 64, 64
EPS_BN, EPS_LN = 1e-5, 1e-6
N1, N2, NS = H1 * W1, H2 * W2, R * R     # 1024, 4096, 256
P = 128
NCORES = 8
PW1, PW2 = W1 + 2, W2 + 2                # padded widths 34, 66
PAD1, PAD2 = (H1 + 2) * PW1, (H2 + 2) * PW2   # 1156, 4356
TAPS = [(dy, dx) for dy in range(3) for dx in range(3)]


# ---------------------------------------------------------------- host helpers

def _interp_matrix(n_in, n_out):
    A = np.zeros((n_out, n_in), np.float32)
    xs = np.linspace(0.0, n_in - 1.0, n_out)
    for i, x in enumerate(xs):
        x0 = int(np.floor(x)); x1 = min(x0 + 1, n_in - 1)
        w = x - x0
        A[i, x0] += 1.0 - w
        A[i, x1] += w
    return A


def _head_major_perm():
    perm = np.zeros(OUT_CH, np.int64)
    for h in range(HEADS):
        for d in range(DIM_HEAD):
            perm[h * DIM_HEAD + d] = d * HEADS + h
    return perm


def _rel_bias_small(rel_table):
    c = np.stack(np.meshgrid(np.arange(R), np.arange(R), indexing="ij")).reshape(2, -1)
    rel = (c[:, :, None] - c[:, None, :]).transpose(1, 2, 0)
    rel[:, :, 0] += R - 1
    rel[:, :, 1] += R - 1
    rel[:, :, 0] *= 2 * R - 1
    idx = rel.sum(-1).reshape(-1)
    return np.asarray(rel_table, np.float32)[idx].reshape(NS, NS, HEADS)


def _r64_chunks():
    """Residue resize (32->64), ch-major: per 512-pixel output chunk only a
    few 128-pixel input tiles contribute."""
    Ay, Ax = _interp_matrix(H1, H2), _interp_matrix(W1, W2)
    R64 = np.kron(Ay, Ax).astype(np.float32)       # [4096, 1024]
    ktiles, blocks = [], []
    for nn in range(8):
        rows = R64[nn * 512:(nn + 1) * 512]
        used = [kk for kk in range(8)
                if np.abs(rows[:, kk * 128:(kk + 1) * 128]).sum() > 0]
        ktiles.append(used)
        for kk in used:
            blocks.append(rows[:, kk * 128:(kk + 1) * 128].T.copy())
    return ktiles, np.concatenate(blocks, axis=0)


_R64_KTILES, _R64_PACKED = _r64_chunks()
_N_R64_SLOTS = sum(len(k) for k in _R64_KTILES)


def _dw_ext(dw):
    """Extend a [C, 9] depthwise tap table with derived columns:
    col 9: W9 (full sum), 10: r0 (top row sum), 11: r2 (bottom row),
    12: c0 (left col), 13: c2 (right col),
    14-17: corner taps w00, w02, w20, w22."""
    C = dw.shape[0]
    e = np.zeros((C, 18), np.float32)
    e[:, 0:9] = dw
    e[:, 9] = dw.sum(1)
    e[:, 10] = dw[:, 0] + dw[:, 1] + dw[:, 2]
    e[:, 11] = dw[:, 6] + dw[:, 7] + dw[:, 8]
    e[:, 12] = dw[:, 0] + dw[:, 3] + dw[:, 6]
    e[:, 13] = dw[:, 2] + dw[:, 5] + dw[:, 8]
    e[:, 14] = dw[:, 0]
    e[:, 15] = dw[:, 2]
    e[:, 16] = dw[:, 6]
    e[:, 17] = dw[:, 8]
    return e


def _dw_diag(dw1, dwq, dwo):
    """Diagonal-weight matrices for the PE-side taps 1..4 of all three
    depthwise convs. Slot layout: DW1 tiles 0-3 (slots 0-15), DWQ tiles
    0-1 (16-23), DWO tiles 0-1 (24-31): [32*128, 128]."""
    out = np.zeros((32, P, P), np.float32)
    s = 0
    for tbl, ntiles in ((dw1, 4), (dwq, 2), (dwo, 2)):
        for tt in range(ntiles):
            for j in range(4):
                np.fill_diagonal(out[s], tbl[tt * P:(tt + 1) * P, 1 + j])
                s += 1
    return out.reshape(32 * P, P)


def _host_prep(inp):
    perm = _head_major_perm()
    f32 = lambda a: np.ascontiguousarray(np.asarray(a, np.float32))
    bf = lambda a: np.ascontiguousarray(np.asarray(a, np.float32).astype(ml_dtypes.bfloat16))

    kvw = np.asarray(inp["to_kv_pw"], np.float32).reshape(2 * OUT_CH, IN_CH)
    selb = np.zeros((16 * P, OUT_CH), np.float32)
    hh = np.arange(OUT_CH) // DIM_HEAD
    for blk in range(16):
        for h in range(HEADS):
            # stats row packing (set by the relayout DMA stream order):
            # row = 16*(blk//2) + 2h + (blk%2)
            selb[blk * P + 16 * (blk // 2) + 2 * h + (blk % 2), :] = (hh == h)

    gq = np.asarray(inp["normq_g"], np.float32).reshape(-1)   # head-major (h,d)
    wq = np.asarray(inp["to_q_pw"], np.float32).reshape(OUT_CH, OUT_CH)[perm].T
    wq = wq * gq[None, :]                                     # fold g into WQ
    selq1 = np.equal(hh[:, None], np.arange(HEADS)[None, :]).astype(np.float32)
    selq1 = selq1 / gq[:, None]
    selq2 = selq1 / gq[:, None]

    d = {
        "wch": f32(np.asarray(inp["conv_ch_w"], np.float32).reshape(OUT_CH, IN_CH).T),
        "wkv": bf(np.concatenate([kvw[perm].T, kvw[OUT_CH + perm].T], axis=1)),
        "wq": bf(wq),
        "wout": bf(np.asarray(inp["to_out_pw"], np.float32).reshape(OUT_CH, OUT_CH)[:, perm].T),
        "wmlp": bf(np.asarray(inp["mlp_w"], np.float32).reshape(OUT_CH, OUT_CH).T),
        "dw1w": f32(_dw_ext(np.asarray(inp["to_kv_dw"], np.float32).reshape(IN_CH, 9))),
        "dwqw": f32(_dw_ext(np.asarray(inp["to_q_dw"], np.float32).reshape(OUT_CH, 9))),
        "dwow": f32(np.asarray(inp["to_out_dw"], np.float32).reshape(OUT_CH, 9)[perm]),
        "dwod": bf(_dw_diag(
            np.asarray(inp["to_kv_dw"], np.float32).reshape(IN_CH, 9),
            np.asarray(inp["to_q_dw"], np.float32).reshape(OUT_CH, 9),
            np.asarray(inp["to_out_dw"], np.float32).reshape(OUT_CH, 9)[perm])),
        "idm": bf(np.eye(P, dtype=np.float32)),
        "rt16": bf(np.kron(_interp_matrix(H1, R), _interp_matrix(W1, R)).T),
        "r64c": bf(_R64_PACKED),
        "selq1": bf(selq1),
        "selq2": bf(selq2),
        "selb": bf(selb),
        "bvt": bf(_rel_bias_small(inp["rel_table"]).transpose(2, 1, 0)
                  .reshape(HEADS * NS, NS)),
        "gkb": bf(np.tile(np.asarray(inp["normk_g"], np.float32).reshape(1, OUT_CH), (P, 1))),
        "bkb": bf(np.tile(np.asarray(inp["normk_b"], np.float32).reshape(1, OUT_CH), (P, 1))),
        "gqb": bf(gq.reshape(OUT_CH, 1)),
        "bqb": bf(np.asarray(inp["normq_b"], np.float32).reshape(OUT_CH, 1)),
    }
    pk = np.zeros((P, 18), np.float32)
    pk[:, 0:4] = np.asarray(inp["norm_l_g"], np.float32).reshape(4, P).T
    pk[:, 4:8] = np.asarray(inp["norm_l_b"], np.float32).reshape(4, P).T
    pk[:, 8:10] = np.asarray(inp["norm_h_g"], np.float32).reshape(2, P).T
    pk[:, 10:12] = np.asarray(inp["norm_h_b"], np.float32).reshape(2, P).T
    pk[:, 12:14] = np.asarray(inp["norm2_g"], np.float32).reshape(2, P).T
    pk[:, 14:16] = np.asarray(inp["norm2_b"], np.float32).reshape(2, P).T
    pk[:, 16:18] = np.asarray(inp["conv_ch_b"], np.float32).reshape(2, P).T
    d["bnpk"] = pk
    return d


# ---------------------------------------------------------------- device build

def _img(ap, w):
    return ap.rearrange("p (h w) -> p h w", w=w)


def _emit_dw_pe_taps(nc, pspool, diag, dslot, xpad, Hs, Ws, pw, name):
    """Taps 1..4 on the TensorEngine: diagonal-weight matmuls accumulated
    in PSUM over shifted padded-image views. Returns the PSUM tiles, one
    per 512-pixel output chunk (tap-outer loop reuses LDWEIGHTS)."""
    nch = (Hs * Ws) // 512
    rpc = 512 // Ws                     # rows per 512-px chunk
    xv = _img(xpad, pw)
    paccs = []
    for w0 in range(0, nch, 4):
        cs = list(range(w0, min(w0 + 4, nch)))
        accs = [pspool.tile([P, 512], FP32, tag="mm512", name=f"{name}{c}")
                for c in cs]
        for j in range(4):
            dy, dx = TAPS[1 + j]
            for a, c in zip(accs, cs):
                nc.tensor.matmul(a[:],
                                 diag[:, dslot + j, :],
                                 xv[:, dy + rpc * c:dy + rpc * c + rpc,
                                    dx:dx + Ws],
                                 start=(j == 0), stop=(j == 3))
        paccs += accs
    return paccs


def _emit_dw_taps(nc, tmps, out, xpad, wvec, Hs, Ws, pw, n_scalar=4,
                  pe=False):
    """Non-PE taps. With pe=False: DVE tensor_scalar tap0 + STT taps
    1..(8-n_scalar), Scalar copies the last n_scalar taps. With pe=True
    (taps 1-4 done by _emit_dw_pe_taps): DVE tap0 (+ STT for taps
    5+n_scalar..8), Scalar taps 5..4+n_scalar. GpSimd is deliberately NOT
    used: its big TT ops saturate SBUF bandwidth and slow concurrent DVE
    streams ~3x (measured)."""
    dst = _img(out, Ws)
    xv = _img(xpad, pw)
    src = lambda i: xv[:, TAPS[i][0]:TAPS[i][0] + Hs, TAPS[i][1]:TAPS[i][1] + Ws]
    nc.vector.tensor_scalar(dst, src(0), wvec[:, 0:1], None, ALU.mult)
    base = 5 if pe else 9 - n_scalar
    dve_taps = (range(5 + n_scalar, 9) if pe else range(1, 9 - n_scalar))
    for i in dve_taps:
        nc.vector.scalar_tensor_tensor(dst, src(i), wvec[:, i:i + 1], dst,
                                       ALU.mult, ALU.add)
    for j in range(n_scalar):
        i = base + j
        nc.scalar.activation(_img(tmps[j][:], Ws), src(i), ACTF.Identity,
                             bias=0.0, scale=wvec[:, i:i + 1])


def _emit_dw_merge(nc, tmps, out, paccs, n_scalar=4):
    """DVE folds the PE-tap PSUM partials first (frees PSUM banks so the
    next wave's matmuls aren't stalled behind the scalar copies), then
    pair-trees the scalar tap copies into out (bf16 2x TT)."""
    for c, a in enumerate(paccs):
        nc.vector.tensor_add(out[:, 512 * c:512 * c + 512],
                             out[:, 512 * c:512 * c + 512], a[:])
    for j in range(n_scalar // 2):
        nc.vector.tensor_add(tmps[2 * j][:], tmps[2 * j][:], tmps[2 * j + 1][:])
    for j in range(n_scalar // 2):
        nc.vector.tensor_add(out, out, tmps[2 * j][:])
    if n_scalar % 2:
        nc.vector.tensor_add(out, out, tmps[n_scalar - 1][:])


def _emit_dw_affine(nc, out, bnS, ew0, ewe, ewc, Hs, Ws):
    """Apply BN affine after a raw depthwise: out = s*out + t*W9 interior,
    with border corrections. All DVE: the main pass is one two-scalar
    tensor_scalar (4x mode), the 8 border fixes are tiny TS adds.
    ew0 [p,1] = t*W9; ewe [p,4] = -t*[r0,r2,c0,c2]; ewc [p,4] = t*corner taps."""
    nc.vector.tensor_scalar(out, out, bnS, ew0, ALU.mult, ALU.add)
    v = _img(out, Ws)
    N = Hs * Ws
    ts = lambda ap, b: nc.vector.tensor_scalar(ap, ap, b, None, ALU.add)
    ts(out[:, 0:Ws], ewe[:, 0:1])                  # top row: -t*r0
    ts(out[:, N - Ws:N], ewe[:, 1:2])              # bottom:  -t*r2
    ts(v[:, :, 0:1], ewe[:, 2:3])                  # left col: -t*c0
    ts(v[:, :, Ws - 1:Ws], ewe[:, 3:4])            # right:    -t*c2
    # corners: add back the doubly-subtracted corner tap
    ts(v[:, 0, 0:1], ewc[:, 0:1])
    ts(v[:, 0, Ws - 1:Ws], ewc[:, 1:2])
    ts(v[:, Hs - 1, 0:1], ewc[:, 2:3])
    ts(v[:, Hs - 1, Ws - 1:Ws], ewc[:, 3:4])


def _pad_memsets(nc, xpad, Hs, Ws, pw):
    """Zero only the pad cells of a [p, (Hs+2)*pw] image buffer (GpSimd)."""
    # top row + leading left-pad cell of first interior row
    nc.gpsimd.memset(xpad[:, 0:pw + 1], 0.0)
    # per interior row: trailing right-pad + next row's left-pad (2 cells,
    # adjacent because pw == Ws + 2)
    mid = xpad[:, pw + Ws + 1:pw + Ws + 1 + (Hs - 1) * pw] \
        .rearrange("p (h c) -> p h c", c=pw)[:, :, 0:2]
    nc.gpsimd.memset(mid, 0.0)
    # bottom pad row + trailing right-pad cell of last interior row
    nc.gpsimd.memset(xpad[:, (Hs + 1) * pw - 1:(Hs + 2) * pw], 0.0)


def _emit(nc, tc, dram, out_d):
    import contextlib
    ctx = contextlib.ExitStack()
    pool = lambda name, bufs, space="SBUF": ctx.enter_context(
        tc.tile_pool(name=name, bufs=bufs, space=space))

    consts = pool("consts", 1)
    work = pool("work", 1)       # unique-tag persistents (small)
    big32 = pool("big32", 1)     # X2 early / OSB late (32KB fp32 class)
    bigA = pool("bigA", 1)       # XP2 early / OPAD late (17.4KB bf16)
    bigB = pool("bigB", 1)       # DWQraw early / DWO late (16KB bf16)
    bigC = pool("bigC", 1)       # X1 fp32 early / Q + RELU later
    big16 = pool("big16", 1)     # remaining 16KB-class persists
    tmpp = pool("tmpp", 1)       # tap tmp buffers [P, N2] bf16 (4 tags)
    tr4 = pool("tr4", 3)         # transient ~4KB chunks
    ps = pool("ps", 4, "PSUM")
    pss = pool("pss", 2, "PSUM")
    dpool = pool("dramp", 1, "DRAM")

    dma = nc.sync.dma_start

    # ---------------- inputs / constants
    X1 = bigC.tile([P, 4, N1], FP32, tag="bigC")
    dma(X1[:], dram["x1"].ap().rearrange("(t p) n -> p t n", p=P))

    def load_c(name, shape, dt=FP32):
        t = consts.tile(shape, dt, tag=name)
        src = dram[name].ap()
        if len(shape) == 3:
            src = src.rearrange("(t p) n -> p t n", p=P)
        dma(t[:], src)
        return t

    # order matters: small tiles that gate early work load first
    DW1W = load_c("dw1w", [P, 4, 18])
    DWQW = load_c("dwqw", [P, 2, 18])
    DWOW = load_c("dwow", [P, 2, 9])
    BNPK = load_c("bnpk", [P, 18])
    X2 = big32.tile([P, 2, N2], FP32, tag="big32")
    dma(X2[:], dram["x2"].ap().rearrange("(t p) n -> p t n", p=P))
    WCH = load_c("wch", [P, 4, OUT_CH])
    WKV = load_c("wkv", [P, 4, 2 * OUT_CH], BF16)
    RT16 = load_c("rt16", [P, 8, NS], BF16)
    WQ = load_c("wq", [P, 2, OUT_CH], BF16)
    SELQ1 = load_c("selq1", [P, 2, HEADS], BF16)
    SELQ2 = load_c("selq2", [P, 2, HEADS], BF16)
    GKB = load_c("gkb", [P, OUT_CH], BF16)
    BKB = load_c("bkb", [P, OUT_CH], BF16)
    GQB = load_c("gqb", [P, 2, 1], BF16)
    BQB = load_c("bqb", [P, 2, 1], BF16)
    BVT = load_c("bvt", [P, 2 * HEADS, NS], BF16)
    SELB = load_c("selb", [P, 16, OUT_CH], BF16)
    WOUT = load_c("wout", [P, 2, OUT_CH], BF16)
    WMLP = load_c("wmlp", [P, 2, OUT_CH], BF16)

    IDM = load_c("idm", [P, P], BF16)
    # diag-tap tables for the PE-side taps of DWQ (slots 0-7) and DWO (8-15)
    DIAG = tmpp.tile([P, 16, P], BF16, tag="tmp4", name="dwdiag")
    dma(DIAG[:], dram["dwod"].ap().rearrange("(t p) n -> p t n", p=P)[:, 16:32, :])

    # ---------------- padded raw images + BN stats (Scalar engine, fused)
    XP1 = [work.tile([P, PAD1], BF16, tag=f"XP1_{t}", name=f"XP1_{t}")
           for t in range(4)]
    XP2 = bigA.tile([P, 2, PAD2], BF16, tag="bigA")
    for t in range(4):
        _pad_memsets(nc, XP1[t][:], H1, W1, PW1)
    for t in range(2):
        _pad_memsets(nc, XP2[:, t, :], H2, W2, PW2)

    # layout: 0..7 x1 (S,S2)x4; 8,9 x2 S; 10..13 x2-t0 S2 chunks; 14..17 t1
    ccin = work.tile([P, 18], FP32, tag="ccin")
    trash = work.tile([P, N1], BF16, tag="trash")
    # x1 stats on DVE (idle at the head) so the scalar queue reaches the
    # x2 stats -- which gate the collective -- sooner
    st1 = work.tile([P, 4, 12], FP32, tag="st1")
    ag1 = work.tile([P, 4, 2], FP32, tag="ag1")
    for t in range(4):
        for c in range(2):
            nc.vector.bn_stats(st1[:, t, 6 * c:6 * c + 6], X1[:, t, bass.ts(c, 512)])
        nc.vector.bn_aggr(ag1[:, t, :],
                          st1[:, t, :].rearrange("p (c s) -> p c s", s=6))
        m, v = ag1[:, t, 0:1], ag1[:, t, 1:2]
        S, S2 = ccin[:, 2 * t:2 * t + 1], ccin[:, 2 * t + 1:2 * t + 2]
        nc.vector.tensor_scalar(S, m, float(N1), None, ALU.mult)
        nc.vector.tensor_mul(S2, m, m)
        nc.vector.tensor_add(S2, S2, v)
        nc.vector.tensor_scalar(S2, S2, float(N1), None, ALU.mult)
    for t in range(4):
        nc.scalar.activation(_img(XP1[t][:], PW1)[:, 1:1 + H1, 1:1 + W1],
                             _img(X1[:, t, :], W1), ACTF.Identity, bias=0.0)
    for t in range(2):
        nc.scalar.activation(_img(XP2[:, t, :], PW2)[:, 1:1 + H2, 1:1 + W2],
                             _img(X2[:, t, :], W2), ACTF.Identity, bias=0.0,
                             accum_out=ccin[:, 8 + t:9 + t])
        for c in range(4):
            nc.scalar.activation(trash[:], X2[:, t, bass.ts(c, N1)], ACTF.Square,
                                 accum_out=ccin[:, 10 + 4 * t + c:11 + 4 * t + c])

    # collective input DMA dispatched from the (idle) GpSimd queue so it
    # doesn't sit behind the const loads on the Sync queue
    cc1i = dpool.tile([P, 18], FP32, tag="cc1i")
    cc1o = dpool.tile([P, 18], FP32, tag="cc1o")
    nc.gpsimd.dma_start(cc1i[:], ccin[:])
    nc.gpsimd.collective_compute("AllReduce", ALU.add,
                                 replica_groups=[list(range(NCORES))],
                                 ins=[cc1i.opt()], outs=[cc1o.opt()])
    ccout = work.tile([P, 18], FP32, tag="ccout")

    # ---------------- conv_ch transposed (for the residue, consumed late)
    X1CT = work.tile([P, 8, OUT_CH], BF16, tag="X1CT")
    for m in range(8):
        acc = ps.tile([P, 512], FP32, tag="mm512")
        for kk in range(4):
            nc.tensor.matmul(acc[:, 0:OUT_CH], X1[:, kk, bass.ts(m, P)],
                             WCH[:, kk, :], start=(kk == 0), stop=(kk == 3))
        nc.scalar.copy(X1CT[:, m, :], acc[:, 0:OUT_CH])

    # ---------------- raw depthwise (runs during the AllReduce flight)
    # taps 1-4 of every depthwise tile run on the TensorEngine (idle in this
    # window) as diagonal matmuls; tap 0 is a DVE tensor_scalar; taps 5-8
    # split between DVE STT and Scalar copies depending on who is free
    DW1 = work.tile([P, 4, N1], BF16, tag="DW1")
    tmps1 = [tmpp.tile([P, N2], BF16, tag=f"tmp{j}", name=f"tmp{j}")
             for j in range(5)]
    for t in range(4):
        ns = 0 if t < 2 else 4
        sub = [SubTile(tt, N1) for tt in tmps1]
        _emit_dw_taps(nc, sub, DW1[:, t, :], XP1[t][:], DW1W[:, t, :],
                      H1, W1, PW1, n_scalar=ns)
        _emit_dw_merge(nc, sub, DW1[:, t, :], [], n_scalar=ns)
    DWQ = bigB.tile([P, 2, N2], BF16, tag="bigB")

    # ---------------- BN scale/shift sandwich (DVE reaches this right as
    # the collective returns); ccout fetched from the DVE queue
    nc.scalar.dma_start(ccout[:], cc1o[:])
    bnS = work.tile([P, 6], FP32, tag="bnS")
    bnT = work.tile([P, 6], FP32, tag="bnT")
    mean6 = work.tile([P, 6], FP32, tag="mean6")
    var6 = work.tile([P, 6], FP32, tag="var6")
    s2x2 = work.tile([P, 2], FP32, tag="s2x2")
    for t in range(2):
        nc.vector.tensor_reduce(s2x2[:, t:t + 1],
                                ccout[:, 10 + 4 * t:14 + 4 * t],
                                mybir.AxisListType.X, ALU.add, opt_input=False)
    for t in range(6):
        n = float(B * (N1 if t < 4 else N2))
        if t < 4:
            S, S2 = ccout[:, 2 * t:2 * t + 1], ccout[:, 2 * t + 1:2 * t + 2]
        else:
            S, S2 = ccout[:, 8 + (t - 4):9 + (t - 4)], s2x2[:, t - 4:t - 3]
        m, v = mean6[:, t:t + 1], var6[:, t:t + 1]
        nc.vector.tensor_scalar(m, S, 1.0 / n, None, ALU.mult)
        nc.vector.scalar_tensor_tensor(v, m, -1.0, m, ALU.mult, ALU.mult)
        nc.vector.scalar_tensor_tensor(v, S2, 1.0 / n, v, ALU.mult, ALU.add)
        nc.vector.tensor_scalar(v, v, EPS_BN, None, ALU.add)
    nc.vector.reciprocal(var6[:], var6[:])
    nc.scalar.activation(bnS[:], var6[:], ACTF.Sqrt)
    nc.vector.tensor_mul(bnS[:, 0:4], bnS[:, 0:4], BNPK[:, 0:4])
    nc.vector.tensor_mul(bnS[:, 4:6], bnS[:, 4:6], BNPK[:, 8:10])
    nc.vector.tensor_mul(mean6[:], mean6[:], bnS[:])
    nc.vector.tensor_sub(bnT[:, 0:4], BNPK[:, 4:8], mean6[:, 0:4])
    nc.vector.tensor_sub(bnT[:, 4:6], BNPK[:, 10:12], mean6[:, 4:6])

    # edge scalars: ew0 = t*W9; ewe = -t*[r0,r2,c0,c2]; ewc = t*corners
    negT = work.tile([P, 6], FP32, tag="negT")
    nc.vector.tensor_scalar(negT[:], bnT[:], -1.0, None, ALU.mult)
    EW01 = work.tile([P, 4, 1], FP32, tag="EW01")
    EWE1 = work.tile([P, 4, 4], FP32, tag="EWE1")
    EWC1 = work.tile([P, 4, 4], FP32, tag="EWC1")
    EW0Q = work.tile([P, 2, 1], FP32, tag="EW0Q")
    EWEQ = work.tile([P, 2, 4], FP32, tag="EWEQ")
    EWCQ = work.tile([P, 2, 4], FP32, tag="EWCQ")
    for t in range(4):
        nc.vector.tensor_scalar(EW01[:, t, :], DW1W[:, t, 9:10],
                                bnT[:, t:t + 1], None, ALU.mult)
        nc.vector.tensor_scalar(EWE1[:, t, :], DW1W[:, t, 10:14],
                                negT[:, t:t + 1], None, ALU.mult)
        nc.vector.tensor_scalar(EWC1[:, t, :], DW1W[:, t, 14:18],
                                bnT[:, t:t + 1], None, ALU.mult)
    for t in range(2):
        nc.vector.tensor_scalar(EW0Q[:, t, :], DWQW[:, t, 9:10],
                                bnT[:, 4 + t:5 + t], None, ALU.mult)
        nc.vector.tensor_scalar(EWEQ[:, t, :], DWQW[:, t, 10:14],
                                negT[:, 4 + t:5 + t], None, ALU.mult)
        nc.vector.tensor_scalar(EWCQ[:, t, :], DWQW[:, t, 14:18],
                                bnT[:, 4 + t:5 + t], None, ALU.mult)

    # first DWQ tile (scalar tap copies already queued ahead of the affines)
    paccq0 = _emit_dw_pe_taps(nc, ps, DIAG, 0, XP2[:, 0, :],
                              H2, W2, PW2, "dqp0_")
    _emit_dw_taps(nc, tmps1, DWQ[:, 0, :], XP2[:, 0, :], DWQW[:, 0, :],
                  H2, W2, PW2, n_scalar=4, pe=True)
    _emit_dw_merge(nc, tmps1, DWQ[:, 0, :], paccq0, n_scalar=4)

    # apply BN affine to DW1 (Scalar engine only), then kv pointwise can start
    for t in range(4):
        _emit_dw_affine(nc, DW1[:, t, :], bnS[:, t:t + 1], EW01[:, t, :],
                        EWE1[:, t, :], EWC1[:, t, :], H1, W1)

    # ---------------- kv pointwise (pixel-major out)
    KVT = big16.tile([P, 8, 2 * OUT_CH], BF16, tag="big16")
    for m in range(8):
        acc = ps.tile([P, 512], FP32, tag="mm512")
        for kk in range(4):
            nc.tensor.matmul(acc[:], DW1[:, kk, bass.ts(m, P)], WKV[:, kk, :],
                             start=(kk == 0), stop=(kk == 3))
        nc.vector.tensor_copy(KVT[:, m, :], acc[:])

    # second DWQ tile + both affines
    paccq1 = _emit_dw_pe_taps(nc, ps, DIAG, 4, XP2[:, 1, :],
                              H2, W2, PW2, "dqp1_")
    _emit_dw_taps(nc, tmps1, DWQ[:, 1, :], XP2[:, 1, :], DWQW[:, 1, :],
                  H2, W2, PW2, n_scalar=4, pe=True)
    _emit_dw_merge(nc, tmps1, DWQ[:, 1, :], paccq1, n_scalar=4)
    for t in range(2):
        _emit_dw_affine(nc, DWQ[:, t, :], bnS[:, 4 + t:5 + t], EW0Q[:, t, :],
                        EWEQ[:, t, :], EWCQ[:, t, :], H2, W2)

    # resize 32->16: kvsT = RT16^T @ KVT  [256 smallpix, 512]
    KVS = []
    for mm in range(2):
        acc = pss.tile([P, 512], FP32, tag="psmall")
        for kk in range(8):
            nc.tensor.matmul(acc[:], RT16[:, kk, bass.ts(mm, P)], KVT[:, kk, :],
                             start=(kk == 0), stop=(kk == 7))
        KVS.append(acc)

    # LN-k + evac k' ; v' plain evac (bf16)
    KP = work.tile([P, 2, OUT_CH], BF16, tag="KP")
    VP = work.tile([P, 2, OUT_CH], BF16, tag="VP")
    ksq = work.tile([P, OUT_CH], FP32, tag="ksq")
    ksum = work.tile([P, HEADS], FP32, tag="ksum")
    km = work.tile([P, HEADS], FP32, tag="km")
    krs = work.tile([P, HEADS], FP32, tag="krs")
    kfp = work.tile([P, OUT_CH], FP32, tag="kfp")
    for mm in range(2):
        k_ap = KVS[mm][:, 0:OUT_CH].rearrange("p (h d) -> p h d", d=DIM_HEAD)
        nc.vector.tensor_reduce(ksum[:], k_ap, mybir.AxisListType.X, ALU.add,
                                opt_input=False)
        nc.scalar.activation(ksq[:], KVS[mm][:, 0:OUT_CH], ACTF.Square)
        nc.vector.tensor_reduce(krs[:], ksq[:].rearrange("p (h d) -> p h d",
                                                         d=DIM_HEAD),
                                mybir.AxisListType.X, ALU.add, opt_input=False)
        nc.vector.scalar_tensor_tensor(km[:], ksum[:], -1.0 / DIM_HEAD, ksum[:],
                                       ALU.mult, ALU.mult)
        nc.vector.tensor_add(krs[:], krs[:], km[:])
        nc.vector.tensor_scalar(krs[:], krs[:], DIM_HEAD * EPS_LN, None, ALU.add)
        nc.vector.reciprocal(krs[:], krs[:])
        nc.scalar.activation(krs[:], krs[:], ACTF.Sqrt, scale=float(DIM_HEAD))
        nc.vector.tensor_scalar(km[:], ksum[:], 1.0 / DIM_HEAD, None, ALU.mult)
        kb = km[:].unsqueeze(2).broadcast_to([P, HEADS, DIM_HEAD])
        rb = krs[:].unsqueeze(2).broadcast_to([P, HEADS, DIM_HEAD])
        t1 = kfp[:].rearrange("p (h d) -> p h d", d=DIM_HEAD)
        nc.vector.tensor_sub(t1, k_ap, kb)
        nc.vector.tensor_mul(t1, t1, rb)
        nc.vector.tensor_mul(kfp[:], kfp[:], GKB[:])
        nc.vector.tensor_add(KP[:, mm, :], kfp[:], BKB[:])
        nc.vector.tensor_copy(VP[:, mm, :], KVS[mm][:, OUT_CH:2 * OUT_CH])

    # A = K'^T V' / 32 : diagonal head blocks packed block-diagonal
    BD = work.tile([P, 2, P], BF16, tag="BD")
    nc.gpsimd.memset(BD[:], 0.0)
    for mo in range(2):
        acc = pss.tile([P, 512], FP32, tag="psmall")
        for kk in range(2):
            nc.tensor.matmul(acc[:, 0:OUT_CH], KP[:, kk, bass.ts(mo, P)],
                             VP[:, kk, :], start=(kk == 0), stop=(kk == 1))
        for hh in range(4):
            h = mo * 4 + hh
            nc.scalar.activation(BD[bass.ds(32 * hh, 32), mo, bass.ds(32 * hh, 32)],
                                 acc[bass.ds(32 * hh, 32), bass.ds(32 * h, 32)],
                                 ACTF.Copy, scale=1.0 / DIM_HEAD)

    # Bb = BD @ b_q, Gg = BD @ g (per-channel consts for deferred LN-q)
    BbGg = work.tile([P, 2, 2], FP32, tag="BbGg")   # [:, pk, 0]=Bb, 1=-Gg
    for pk in range(2):
        acc = pss.tile([P, 512], FP32, tag="psmall")
        nc.tensor.matmul(acc[:, 0:1], BD[:, pk, :], BQB[:, pk, :],
                         start=True, stop=True)
        nc.tensor.matmul(acc[:, 1:2], BD[:, pk, :], GQB[:, pk, :],
                         start=True, stop=True)
        nc.scalar.copy(BbGg[:, pk, 0:1], acc[:, 0:1])
        nc.scalar.activation(BbGg[:, pk, 1:2], acc[:, 1:2], ACTF.Identity,
                             bias=0.0, scale=-1.0)

    # BV[(h,d'), is] = (v'^T bias_small_h)/32 via full-M matmul + row extract
    BVC = work.tile([P, 2, NS], BF16, tag="BVC")
    for h in range(HEADS):
        mo, hh = h // 4, h % 4
        acc = pss.tile([P, 512], FP32, tag="psmall")
        for kk in range(2):
            nc.tensor.matmul(acc[:, 0:NS], VP[:, kk, bass.ts(mo, P)],
                             BVT[:, 2 * h + kk, :], start=(kk == 0), stop=(kk == 1))
        nc.scalar.activation(BVC[bass.ds(32 * hh, 32), mo, :],
                             acc[bass.ds(32 * hh, 32), 0:NS],
                             ACTF.Copy, scale=1.0 / DIM_HEAD)
    # expand along x: BVX[c, ys*64 + x] = BVC[c, ys*16 + x//4]; then += Bb
    BVX = work.tile([P, 2, R * W2], BF16, tag="BVX")
    for mo in range(2):
        nc.vector.tensor_copy(
            BVX[:, mo, :].rearrange("p (ys xs xr) -> p ys xs xr", xs=R, xr=4),
            BVC[:, mo, :].rearrange("p (ys xs) -> p ys xs", xs=R)
            .unsqueeze(3).broadcast_to([P, R, R, 4]))
        nc.vector.tensor_scalar(BVX[:, mo, :], BVX[:, mo, :],
                                BbGg[:, mo, 0:1], None, ALU.add)

    # ---------------- q pointwise (g-folded) + LN-q stats
    Q = bigC.tile([P, 2, N2], BF16, tag="bigC")     # reuses X1 slot
    QSP = work.tile([P, 2, NS], FP32, tag="QSP")   # [(h*16+blk), (qs|q2s), 256]
    for nn in range(8):
        q2c = tr4.tile([P, 2, 512], BF16, tag="tr4")
        for mm in range(2):
            acc = ps.tile([P, 512], FP32, tag="mm512")
            for kk in range(2):
                nc.tensor.matmul(acc[:], WQ[:, kk, bass.ts(mm, P)],
                                 DWQ[:, kk, bass.ts(nn, 512)],
                                 start=(kk == 0), stop=(kk == 1))
            nc.scalar.copy(Q[:, mm, bass.ts(nn, 512)], acc[:])
            nc.vector.tensor_mul(q2c[:, mm, :], Q[:, mm, bass.ts(nn, 512)],
                                 Q[:, mm, bass.ts(nn, 512)])
        for s in range(2):
            sacc = pss.tile([P, 512], FP32, tag="psmall")
            SEL = SELQ1 if s == 0 else SELQ2
            for mm in range(2):
                rhs = Q[:, mm, bass.ts(nn, 512)] if s == 0 else q2c[:, mm, :]
                nc.tensor.matmul(sacc[0:HEADS, :], SEL[:, mm, :], rhs,
                                 start=(mm == 0), stop=(mm == 1))
            qsc = tr4.tile([HEADS, 512], FP32, tag="tr4")
            nc.vector.tensor_copy(qsc[:], sacc[0:HEADS, :])
            # relayout rows: row 16nn + 2h + b  <->  (blk = 2nn+b, h)
            dma(QSP[bass.ds(16 * nn, 16), s, :],
                qsc[:].rearrange("h (b f) -> h b f", f=NS))

    # rs | mrs  (bf16, packed for the broadcast matmul)
    RSP = work.tile([P, 2, NS], BF16, tag="RSP")
    numt = work.tile([P, NS], FP32, tag="numt")
    qsv, q2v = QSP[:, 0, :], QSP[:, 1, :]
    nc.vector.scalar_tensor_tensor(numt[:], qsv, -1.0 / DIM_HEAD, qsv, ALU.mult, ALU.mult)
    nc.vector.tensor_add(numt[:], numt[:], q2v)
    nc.vector.tensor_scalar(numt[:], numt[:], DIM_HEAD * EPS_LN, None, ALU.add)
    nc.vector.reciprocal(numt[:], numt[:])
    nc.scalar.activation(RSP[:, 0, :], numt[:], ACTF.Sqrt, scale=float(DIM_HEAD))
    nc.vector.scalar_tensor_tensor(RSP[:, 1, :], qsv, 1.0 / DIM_HEAD, RSP[:, 0, :],
                                   ALU.mult, ALU.mult)

    # ---------------- per-256-pixel block: broadcast stats, QA matmul on
    # g-folded Q, deferred LN affine on the output, add bias map, write OPAD
    OPAD = bigA.tile([P, 2, PAD2], BF16, tag="bigA")   # reuses XP2 slot
    for t in range(2):
        _pad_memsets(nc, OPAD[:, t, :], H2, W2, PW2)
    rsp_flat = RSP[:].rearrange("p s f -> p (s f)")
    for blk in range(16):
        rsb = tr4.tile([P, 2, 2, NS], BF16, tag="tr4")
        for mm in range(2):
            bacc = pss.tile([P, 512], FP32, tag="psmall")
            nc.tensor.matmul(bacc[:], SELB[:, blk, bass.ts(mm, P)], rsp_flat,
                             start=True, stop=True)
            nc.scalar.copy(rsb[:, mm, :, :],
                           bacc[:].rearrange("p (s f) -> p s f", f=NS))
        for pk in range(2):
            acc = ps.tile([P, 512], FP32, tag="mm512")
            nc.tensor.matmul(acc[:, 0:NS], BD[:, pk, :],
                             Q[:, pk, bass.ds(blk * NS, NS)],
                             start=True, stop=True)
            tmpo = tr4.tile([P, NS], BF16, tag="tr4b")
            nc.vector.tensor_mul(tmpo[:], acc[:, 0:NS], rsb[:, pk, 0, :])
            nc.vector.scalar_tensor_tensor(tmpo[:], rsb[:, pk, 1, :],
                                           BbGg[:, pk, 1:2], tmpo[:],
                                           ALU.mult, ALU.add)
            # rows 4*blk .. 4*blk+4 of the 64x64 image; ys = blk
            dst = _img(OPAD[:, pk, :], PW2)[:, 1 + 4 * blk:5 + 4 * blk, 1:1 + W2]
            bv = BVX[:, pk, bass.ds(blk * W2, W2)].unsqueeze(1) \
                .broadcast_to([P, 4, W2])
            nc.gpsimd.tensor_add(dst,
                                 tmpo[:].rearrange("p (yr w) -> p yr w", w=W2),
                                 bv)

    # ---------------- to_out depthwise + pointwise + residue in PSUM
    # taps 1-4 run on the (otherwise idle) TensorEngine as diagonal-weight
    # matmuls PSUM-accumulated over shifted OPAD views; tap 0 is a DVE
    # tensor_scalar; taps 5-8 are scalar-engine copies merged on DVE.
    DWO = bigB.tile([P, 2, N2], BF16, tag="bigB")   # reuses DWQ slot
    for t in range(2):
        pacco = _emit_dw_pe_taps(nc, ps, DIAG, 8 + 4 * t, OPAD[:, t, :],
                                 H2, W2, PW2, f"dop{t}_")
        _emit_dw_taps(nc, tmps1, DWO[:, t, :], OPAD[:, t, :], DWOW[:, t, :],
                      H2, W2, PW2, n_scalar=4, pe=True)
        _emit_dw_merge(nc, tmps1, DWO[:, t, :], pacco, n_scalar=4)
    # residue-resize table loads into the freed tap-tmp slabs (3 chunks of 8)
    r64src = dram["r64c"].ap().rearrange("(t p) n -> p t n", p=P)
    R64T = []
    for c in range(3):
        lo, hi = 8 * c, min(8 * c + 8, _N_R64_SLOTS)
        rc = tmpp.tile([P, hi - lo, 512], BF16, tag=f"tmp{c}", name=f"r64t{c}")
        dma(rc[:], r64src[:, lo:hi, :])
        R64T.append(rc)
    OSB = big32.tile([P, 2, N2], BF16, tag="big32")   # reuses X2 slot
    # OSB evac (+conv_ch bias) and BN2 stats on DVE -- it is idle during
    # the WOUT matmul phase while the scalar engine was the pacer before
    st2 = work.tile([P, 2, 48], FP32, tag="st2")
    ag2 = work.tile([P, 2, 2], FP32, tag="ag2")
    slot = 0
    for nn in range(8):
        used = _R64_KTILES[nn]
        for mm in range(2):
            acc = ps.tile([P, 512], FP32, tag="mm512")
            for kk in range(2):
                nc.tensor.matmul(acc[:], WOUT[:, kk, bass.ts(mm, P)],
                                 DWO[:, kk, bass.ts(nn, 512)],
                                 start=(kk == 0), stop=False)
            for i, kk in enumerate(used):
                s = slot + i
                nc.tensor.matmul(acc[:], X1CT[:, kk, bass.ts(mm, P)],
                                 R64T[s // 8][:, s % 8, :],
                                 start=False, stop=(i == len(used) - 1))
            nc.vector.tensor_scalar(OSB[:, mm, bass.ts(nn, 512)], acc[:],
                                    BNPK[:, 16 + mm:17 + mm], None, ALU.add)
            nc.vector.bn_stats(st2[:, mm, 6 * nn:6 * nn + 6],
                               OSB[:, mm, bass.ts(nn, 512)])
        slot += len(used)

    # ---------------- BN2 (norm2) stats reduce + AllReduce
    cc2s = work.tile([P, 4], FP32, tag="cc2s")
    for t in range(2):
        nc.vector.bn_aggr(ag2[:, t, :],
                          st2[:, t, :].rearrange("p (c s) -> p c s", s=6))
        m, v = ag2[:, t, 0:1], ag2[:, t, 1:2]
        S, S2 = cc2s[:, 2 * t:2 * t + 1], cc2s[:, 2 * t + 1:2 * t + 2]
        nc.vector.tensor_scalar(S, m, float(N2), None, ALU.mult)
        nc.vector.tensor_mul(S2, m, m)
        nc.vector.tensor_add(S2, S2, v)
        nc.vector.tensor_scalar(S2, S2, float(N2), None, ALU.mult)
    cc2i = dpool.tile([P, 4], FP32, tag="cc2i")
    cc2o = dpool.tile([P, 4], FP32, tag="cc2o")
    nc.gpsimd.dma_start(cc2i[:], cc2s[:])
    nc.gpsimd.collective_compute("AllReduce", ALU.add,
                                 replica_groups=[list(range(NCORES))],
                                 ins=[cc2i.opt()], outs=[cc2o.opt()])
    cc2r = work.tile([P, 4], FP32, tag="cc2r")
    nc.scalar.dma_start(cc2r[:], cc2o[:])
    bn3S = work.tile([P, 2], FP32, tag="bn3S")
    bn3T = work.tile([P, 2], FP32, tag="bn3T")
    m3 = work.tile([P, 2], FP32, tag="m3")
    v3 = work.tile([P, 2], FP32, tag="v3")
    nB = float(B * N2)
    for t in range(2):
        S, S2 = cc2r[:, 2 * t:2 * t + 1], cc2r[:, 2 * t + 1:2 * t + 2]
        nc.vector.tensor_scalar(m3[:, t:t + 1], S, 1.0 / nB, None, ALU.mult)
        nc.vector.scalar_tensor_tensor(v3[:, t:t + 1], m3[:, t:t + 1], -1.0,
                                       m3[:, t:t + 1], ALU.mult, ALU.mult)
        nc.vector.scalar_tensor_tensor(v3[:, t:t + 1], S2, 1.0 / nB,
                                       v3[:, t:t + 1], ALU.mult, ALU.add)
        nc.vector.tensor_scalar(v3[:, t:t + 1], v3[:, t:t + 1], EPS_BN, None, ALU.add)
    nc.vector.reciprocal(v3[:], v3[:])
    nc.scalar.activation(bn3S[:], v3[:], ACTF.Sqrt)
    nc.vector.tensor_mul(bn3S[:], bn3S[:], BNPK[:, 12:14])
    nc.vector.tensor_mul(m3[:], m3[:], bn3S[:])
    nc.vector.tensor_sub(bn3T[:], BNPK[:, 14:16], m3[:])

    # ---------------- relu(bn) + mlp + final residual -> out
    # relu is chunked into the mlp loop so the first matmul starts ~6us
    # earlier after the BN2 collective returns
    RELU = bigC.tile([P, 2, N2], BF16, tag="bigC")   # reuses Q slot
    out_ap = out_d.ap().rearrange("(t p) n -> p t n", p=P)
    for nn in range(8):
        for t in range(2):
            nc.scalar.activation(RELU[:, t, bass.ts(nn, 512)],
                                 OSB[:, t, bass.ts(nn, 512)], ACTF.Relu,
                                 bias=bn3T[:, t:t + 1], scale=bn3S[:, t:t + 1])
        for mm in range(2):
            acc = ps.tile([P, 512], FP32, tag="mm512")
            for kk in range(2):
                nc.tensor.matmul(acc[:], WMLP[:, kk, bass.ts(mm, P)],
                                 RELU[:, kk, bass.ts(nn, 512)],
                                 start=(kk == 0), stop=(kk == 1))
            fin = tr4.tile([P, 512], FP32, tag="tr4")
            nc.vector.tensor_add(fin[:], acc[:], OSB[:, mm, bass.ts(nn, 512)])
            dma(out_ap[:, mm, bass.ts(nn, 512)], fin[:])

    ctx.close()


class SubTile:
    """View adapter: presents the first n columns of a tile as a tile."""
    def __init__(self, t, n):
        self._t = t
        self._n = n

    def __getitem__(self, key):
        return self._t[:, 0:self._n]


def _build_program():
    nc = bacc.Bacc("TRN2", target_bir_lowering=False, debug=False,
                   num_devices=NCORES)
    dram = {}

    def din(name, shape, dt=FP32):
        dram[name] = nc.dram_tensor(name, list(shape), dt, kind="ExternalInput")

    din("x1", (IN_CH, N1)); din("x2", (OUT_CH, N2))
    din("wch", (IN_CH, OUT_CH)); din("wkv", (IN_CH, 2 * OUT_CH), BF16)
    din("wq", (OUT_CH, OUT_CH), BF16); din("wout", (OUT_CH, OUT_CH), BF16)
    din("wmlp", (OUT_CH, OUT_CH), BF16)
    din("dw1w", (IN_CH, 18)); din("dwqw", (OUT_CH, 18)); din("dwow", (OUT_CH, 9))
    din("dwod", (32 * P, P), BF16)
    din("idm", (P, P), BF16)
    din("rt16", (N1, NS), BF16); din("r64c", (_N_R64_SLOTS * P, 512), BF16)
    din("selq1", (OUT_CH, HEADS), BF16); din("selq2", (OUT_CH, HEADS), BF16)
    din("selb", (16 * P, OUT_CH), BF16)
    din("bvt", (HEADS * NS, NS), BF16)
    din("gkb", (P, OUT_CH), BF16); din("bkb", (P, OUT_CH), BF16)
    din("gqb", (OUT_CH, 1), BF16); din("bqb", (OUT_CH, 1), BF16)
    din("bnpk", (P, 18))
    out_d = nc.dram_tensor("out", [OUT_CH, N2], FP32, kind="ExternalOutput")

    with tile.TileContext(nc) as tc:
        _emit(nc, tc, dram, out_d)
    nc.compile()
    return nc


# ------------------------------------------------------------------- run layer

_CACHE = {}


def _get_program():
    if "nc" not in _CACHE:
        _CACHE["nc"] = _build_program()
    return _CACHE["nc"]


def kernel(**inputs):
    nc = _get_program()
    shared = _host_prep(inputs)
    x1 = np.ascontiguousarray(np.asarray(inputs["x1"], np.float32).reshape(B, IN_CH, N1))
    x2 = np.ascontiguousarray(np.asarray(inputs["x2"], np.float32).reshape(B, OUT_CH, N2))
    in_maps = [dict(shared, x1=x1[b], x2=x2[b]) for b in range(B)]
    res = run_bass_kernel_spmd(nc, in_maps, core_ids=list(range(NCORES)))
    out = np.stack([np.asarray(res.results[b]["out"], np.float32)
                    .reshape(OUT_CH, H2, W2) for b in range(B)])
    return out


# revision 70
# speedup vs baseline: 1.2792x; 1.0598x over previous
"""Trainium2 Bass kernel for nn_BasicTransDecoderBlock (dense_transformer).

Strategy: data-parallel over batch B=8 across 8 NeuronCores (1 sample/core).
BatchNorm batch statistics are synchronized with two small AllReduces.
The attention is softmax-free and reassociated:
    O = Q' (K'^T V')/d  +  (bias @ V')/d
which collapses the dominant QK^T/AV FLOPs into tiny per-head d x d matmuls.

v2 optimizations over the first working version:
 - Depthwise convs run on RAW (pre-BN) zero-padded inputs so they can
   execute during the BN-stats AllReduce flight; the BN affine commutes:
   DW(s*x+t) = s*DWraw(x) + t*kappa, applied afterwards on the Scalar
   engine with tiny per-edge border corrections (kappa deviates from the
   full 3x3 weight sum only at image borders).
 - BN statistics are computed with the Scalar engine's accum_out while
   building the padded bf16 images (one fused pass), not with DVE bn_stats.
 - Depthwise taps are split across DVE (scalar_tensor_tensor), Scalar
   (scaled shifted copies) and GpSimd (pair merges) -- STT is hard-capped
   at 1x on DVE so parallel engines beat more DVE work.
 - LayerNorm-q is algebraically deferred: normq_g folds into the Q
   pointwise weights on host, the per-(head,pixel) rs/mrs scalars apply
   AFTER the block-diagonal attention matmul, and BD@b_q pre-adds into
   the bias map.
 - Pad-region-only memsets on GpSimd; PSUM evacuations on Scalar; the
   upsampled residue accumulates directly in the to_out PSUM tile.

Self-contained: hardcodes all shapes; imports only the concourse runtime
shipped in the container.
"""
import sys
import numpy as np
import ml_dtypes

for _p in ("/opt/trn_rl_repo", "/root/.axon_site/_ro/trn_rl_repo"):
    if _p not in sys.path:
        sys.path.insert(0, _p)

import concourse.bass as bass
import concourse.bacc as bacc
import concourse.tile as tile
from concourse import mybir
from concourse.bass_utils import run_bass_kernel_spmd

FP32 = mybir.dt.float32
BF16 = mybir.dt.bfloat16
ALU = mybir.AluOpType
ACTF = mybir.ActivationFunctionType

B, IN_CH, OUT_CH, HEADS, DIM_HEAD, R = 8, 512, 256, 8, 32, 16
H1, W1, H2, W2 = 32, 32, 64, 64
EPS_BN, EPS_LN = 1e-5, 1e-6
N1, N2, NS = H1 * W1, H2 * W2, R * R     # 1024, 4096, 256
P = 128
NCORES = 8
PW1, PW2 = W1 + 2, W2 + 2                # padded widths 34, 66
PAD1, PAD2 = (H1 + 2) * PW1, (H2 + 2) * PW2   # 1156, 4356
TAPS = [(dy, dx) for dy in range(3) for dx in range(3)]


# ---------------------------------------------------------------- host helpers

def _interp_matrix(n_in, n_out):
    A = np.zeros((n_out, n_in), np.float32)
    xs = np.linspace(0.0, n_in - 1.0, n_out)
    for i, x in enumerate(xs):
        x0 = int(np.floor(x)); x1 = min(x0 + 1, n_in - 1)
        w = x - x0
        A[i, x0] += 1.0 - w
        A[i, x1] += w
    return A


def _head_major_perm():
    perm = np.zeros(OUT_CH, np.int64)
    for h in range(HEADS):
        for d in range(DIM_HEAD):
            perm[h * DIM_HEAD + d] = d * HEADS + h
    return perm


def _rel_bias_small(rel_table):
    c = np.stack(np.meshgrid(np.arange(R), np.arange(R), indexing="ij")).reshape(2, -1)
    rel = (c[:, :, None] - c[:, None, :]).transpose(1, 2, 0)
    rel[:, :, 0] += R - 1
    rel[:, :, 1] += R - 1
    rel[:, :, 0] *= 2 * R - 1
    idx = rel.sum(-1).reshape(-1)
    return np.asarray(rel_table, np.float32)[idx].reshape(NS, NS, HEADS)


def _r64_chunks():
    """Residue resize (32->64), ch-major: per 512-pixel output chunk only a
    few 128-pixel input tiles contribute."""
    Ay, Ax = _interp_matrix(H1, H2), _interp_matrix(W1, W2)
    R64 = np.kron(Ay, Ax).astype(np.float32)       # [4096, 1024]
    ktiles, blocks = [], []
    for nn in range(8):
        rows = R64[nn * 512:(nn + 1) * 512]
        used = [kk for kk in range(8)
                if np.abs(rows[:, kk * 128:(kk + 1) * 128]).sum() > 0]
        ktiles.append(used)
        for kk in used:
            blocks.append(rows[:, kk * 128:(kk + 1) * 128].T.copy())
    return ktiles, np.concatenate(blocks, axis=0)


_R64_KTILES, _R64_PACKED = _r64_chunks()
_N_R64_SLOTS = sum(len(k) for k in _R64_KTILES)


def _dw_ext(dw):
    """Extend a [C, 9] depthwise tap table with derived columns:
    col 9: W9 (full sum), 10: r0 (top row sum), 11: r2 (bottom row),
    12: c0 (left col), 13: c2 (right col),
    14-17: corner taps w00, w02, w20, w22."""
    C = dw.shape[0]
    e = np.zeros((C, 18), np.float32)
    e[:, 0:9] = dw
    e[:, 9] = dw.sum(1)
    e[:, 10] = dw[:, 0] + dw[:, 1] + dw[:, 2]
    e[:, 11] = dw[:, 6] + dw[:, 7] + dw[:, 8]
    e[:, 12] = dw[:, 0] + dw[:, 3] + dw[:, 6]
    e[:, 13] = dw[:, 2] + dw[:, 5] + dw[:, 8]
    e[:, 14] = dw[:, 0]
    e[:, 15] = dw[:, 2]
    e[:, 16] = dw[:, 6]
    e[:, 17] = dw[:, 8]
    return e


def _dw_diag(dw1, dwq, dwo):
    """Diagonal-weight matrices for the PE-side taps 1..4 of all three
    depthwise convs. Slot layout: DW1 tiles 0-3 (slots 0-15), DWQ tiles
    0-1 (16-23), DWO tiles 0-1 (24-31): [32*128, 128]."""
    out = np.zeros((32, P, P), np.float32)
    s = 0
    for tbl, ntiles in ((dw1, 4), (dwq, 2), (dwo, 2)):
        for tt in range(ntiles):
            for j in range(4):
                np.fill_diagonal(out[s], tbl[tt * P:(tt + 1) * P, 1 + j])
                s += 1
    return out.reshape(32 * P, P)


def _host_prep(inp):
    perm = _head_major_perm()
    f32 = lambda a: np.ascontiguousarray(np.asarray(a, np.float32))
    bf = lambda a: np.ascontiguousarray(np.asarray(a, np.float32).astype(ml_dtypes.bfloat16))

    kvw = np.asarray(inp["to_kv_pw"], np.float32).reshape(2 * OUT_CH, IN_CH)
    selb = np.zeros((16 * P, OUT_CH), np.float32)
    hh = np.arange(OUT_CH) // DIM_HEAD
    for blk in range(16):
        for h in range(HEADS):
            # stats row packing (set by the relayout DMA stream order):
            # row = 16*(blk//2) + 2h + (blk%2)
            selb[blk * P + 16 * (blk // 2) + 2 * h + (blk % 2), :] = (hh == h)

    gq = np.asarray(inp["normq_g"], np.float32).reshape(-1)   # head-major (h,d)
    wq = np.asarray(inp["to_q_pw"], np.float32).reshape(OUT_CH, OUT_CH)[perm].T
    wq = wq * gq[None, :]                                     # fold g into WQ
    selq1 = np.equal(hh[:, None], np.arange(HEADS)[None, :]).astype(np.float32)
    selq1 = selq1 / gq[:, None]
    selq2 = selq1 / gq[:, None]

    d = {
        "wch": f32(np.asarray(inp["conv_ch_w"], np.float32).reshape(OUT_CH, IN_CH).T),
        "wkv": bf(np.concatenate([kvw[perm].T, kvw[OUT_CH + perm].T], axis=1)),
        "wq": bf(wq),
        "wout": bf(np.asarray(inp["to_out_pw"], np.float32).reshape(OUT_CH, OUT_CH)[:, perm].T),
        "wmlp": bf(np.asarray(inp["mlp_w"], np.float32).reshape(OUT_CH, OUT_CH).T),
        "dw1w": f32(_dw_ext(np.asarray(inp["to_kv_dw"], np.float32).reshape(IN_CH, 9))),
        "dwqw": f32(_dw_ext(np.asarray(inp["to_q_dw"], np.float32).reshape(OUT_CH, 9))),
        "dwow": f32(np.asarray(inp["to_out_dw"], np.float32).reshape(OUT_CH, 9)[perm]),
        "dwod": bf(_dw_diag(
            np.asarray(inp["to_kv_dw"], np.float32).reshape(IN_CH, 9),
            np.asarray(inp["to_q_dw"], np.float32).reshape(OUT_CH, 9),
            np.asarray(inp["to_out_dw"], np.float32).reshape(OUT_CH, 9)[perm])),
        "idm": bf(np.eye(P, dtype=np.float32)),
        "rt16": bf(np.kron(_interp_matrix(H1, R), _interp_matrix(W1, R)).T),
        "r64c": bf(_R64_PACKED),
        "selq1": bf(selq1),
        "selq2": bf(selq2),
        "selb": bf(selb),
        "bvt": bf(_rel_bias_small(inp["rel_table"]).transpose(2, 1, 0)
                  .reshape(HEADS * NS, NS)),
        "gkb": bf(np.tile(np.asarray(inp["normk_g"], np.float32).reshape(1, OUT_CH), (P, 1))),
        "bkb": bf(np.tile(np.asarray(inp["normk_b"], np.float32).reshape(1, OUT_CH), (P, 1))),
        "gqb": bf(gq.reshape(OUT_CH, 1)),
        "bqb": bf(np.asarray(inp["normq_b"], np.float32).reshape(OUT_CH, 1)),
    }
    pk = np.zeros((P, 18), np.float32)
    pk[:, 0:4] = np.asarray(inp["norm_l_g"], np.float32).reshape(4, P).T
    pk[:, 4:8] = np.asarray(inp["norm_l_b"], np.float32).reshape(4, P).T
    pk[:, 8:10] = np.asarray(inp["norm_h_g"], np.float32).reshape(2, P).T
    pk[:, 10:12] = np.asarray(inp["norm_h_b"], np.float32).reshape(2, P).T
    pk[:, 12:14] = np.asarray(inp["norm2_g"], np.float32).reshape(2, P).T
    pk[:, 14:16] = np.asarray(inp["norm2_b"], np.float32).reshape(2, P).T
    pk[:, 16:18] = np.asarray(inp["conv_ch_b"], np.float32).reshape(2, P).T
    d["bnpk"] = pk
    return d


# ---------------------------------------------------------------- device build

def _img(ap, w):
    return ap.rearrange("p (h w) -> p h w", w=w)


def _emit_dw_pe_taps(nc, pspool, diag, dslot, xpad, Hs, Ws, pw, name):
    """Taps 1..4 on the TensorEngine: diagonal-weight matmuls accumulated
    in PSUM over shifted padded-image views. Returns the PSUM tiles, one
    per 512-pixel output chunk (tap-outer loop reuses LDWEIGHTS)."""
    nch = (Hs * Ws) // 512
    rpc = 512 // Ws                     # rows per 512-px chunk
    xv = _img(xpad, pw)
    paccs = []
    for w0 in range(0, nch, 4):
        cs = list(range(w0, min(w0 + 4, nch)))
        accs = [pspool.tile([P, 512], FP32, tag="mm512", name=f"{name}{c}")
                for c in cs]
        for j in range(4):
            dy, dx = TAPS[1 + j]
            for a, c in zip(accs, cs):
                nc.tensor.matmul(a[:],
                                 diag[:, dslot + j, :],
                                 xv[:, dy + rpc * c:dy + rpc * c + rpc,
                                    dx:dx + Ws],
                                 start=(j == 0), stop=(j == 3))
        paccs += accs
    return paccs


def _emit_dw_taps(nc, tmps, out, xpad, wvec, Hs, Ws, pw, n_scalar=4,
                  pe=False):
    """Non-PE taps. With pe=False: DVE tensor_scalar tap0 + STT taps
    1..(8-n_scalar), Scalar copies the last n_scalar taps. With pe=True
    (taps 1-4 done by _emit_dw_pe_taps): DVE tap0 (+ STT for taps
    5+n_scalar..8), Scalar taps 5..4+n_scalar. GpSimd is deliberately NOT
    used: its big TT ops saturate SBUF bandwidth and slow concurrent DVE
    streams ~3x (measured)."""
    dst = _img(out, Ws)
    xv = _img(xpad, pw)
    src = lambda i: xv[:, TAPS[i][0]:TAPS[i][0] + Hs, TAPS[i][1]:TAPS[i][1] + Ws]
    nc.vector.tensor_scalar(dst, src(0), wvec[:, 0:1], None, ALU.mult)
    base = 5 if pe else 9 - n_scalar
    dve_taps = (range(5 + n_scalar, 9) if pe else range(1, 9 - n_scalar))
    for i in dve_taps:
        nc.vector.scalar_tensor_tensor(dst, src(i), wvec[:, i:i + 1], dst,
                                       ALU.mult, ALU.add)
    for j in range(n_scalar):
        i = base + j
        nc.scalar.activation(_img(tmps[j][:], Ws), src(i), ACTF.Identity,
                             bias=0.0, scale=wvec[:, i:i + 1])


def _emit_dw_merge(nc, tmps, out, paccs, n_scalar=4):
    """DVE folds the PE-tap PSUM partials first (frees PSUM banks so the
    next wave's matmuls aren't stalled behind the scalar copies), then
    pair-trees the scalar tap copies into out (bf16 2x TT)."""
    for c, a in enumerate(paccs):
        nc.vector.tensor_add(out[:, 512 * c:512 * c + 512],
                             out[:, 512 * c:512 * c + 512], a[:])
    for j in range(n_scalar // 2):
        nc.vector.tensor_add(tmps[2 * j][:], tmps[2 * j][:], tmps[2 * j + 1][:])
    for j in range(n_scalar // 2):
        nc.vector.tensor_add(out, out, tmps[2 * j][:])
    if n_scalar % 2:
        nc.vector.tensor_add(out, out, tmps[n_scalar - 1][:])


def _emit_dw_affine(nc, out, bnS, ew0, ewe, ewc, Hs, Ws):
    """Apply BN affine after a raw depthwise: out = s*out + t*W9 interior,
    with border corrections. All DVE: the main pass is one two-scalar
    tensor_scalar (4x mode), the 8 border fixes are tiny TS adds.
    ew0 [p,1] = t*W9; ewe [p,4] = -t*[r0,r2,c0,c2]; ewc [p,4] = t*corner taps."""
    nc.vector.tensor_scalar(out, out, bnS, ew0, ALU.mult, ALU.add)
    v = _img(out, Ws)
    N = Hs * Ws
    ts = lambda ap, b: nc.vector.tensor_scalar(ap, ap, b, None, ALU.add)
    ts(out[:, 0:Ws], ewe[:, 0:1])                  # top row: -t*r0
    ts(out[:, N - Ws:N], ewe[:, 1:2])              # bottom:  -t*r2
    ts(v[:, :, 0:1], ewe[:, 2:3])                  # left col: -t*c0
    ts(v[:, :, Ws - 1:Ws], ewe[:, 3:4])            # right:    -t*c2
    # corners: add back the doubly-subtracted corner tap
    ts(v[:, 0, 0:1], ewc[:, 0:1])
    ts(v[:, 0, Ws - 1:Ws], ewc[:, 1:2])
    ts(v[:, Hs - 1, 0:1], ewc[:, 2:3])
    ts(v[:, Hs - 1, Ws - 1:Ws], ewc[:, 3:4])


def _pad_memsets(nc, xpad, Hs, Ws, pw):
    """Zero only the pad cells of a [p, (Hs+2)*pw] image buffer (GpSimd)."""
    # top row + leading left-pad cell of first interior row
    nc.gpsimd.memset(xpad[:, 0:pw + 1], 0.0)
    # per interior row: trailing right-pad + next row's left-pad (2 cells,
    # adjacent because pw == Ws + 2)
    mid = xpad[:, pw + Ws + 1:pw + Ws + 1 + (Hs - 1) * pw] \
        .rearrange("p (h c) -> p h c", c=pw)[:, :, 0:2]
    nc.gpsimd.memset(mid, 0.0)
    # bottom pad row + trailing right-pad cell of last interior row
    nc.gpsimd.memset(xpad[:, (Hs + 1) * pw - 1:(Hs + 2) * pw], 0.0)


def _emit(nc, tc, dram, out_d):
    import contextlib
    ctx = contextlib.ExitStack()
    pool = lambda name, bufs, space="SBUF": ctx.enter_context(
        tc.tile_pool(name=name, bufs=bufs, space=space))

    consts = pool("consts", 1)
    work = pool("work", 1)       # unique-tag persistents (small)
    big32 = pool("big32", 1)     # X2 early / OSB late (32KB fp32 class)
    bigA = pool("bigA", 1)       # XP2 early / OPAD late (17.4KB bf16)
    bigB = pool("bigB", 1)       # DWQraw early / DWO late (16KB bf16)
    bigC = pool("bigC", 1)       # X1 fp32 early / Q + RELU later
    big16 = pool("big16", 1)     # remaining 16KB-class persists
    tmpp = pool("tmpp", 1)       # tap tmp buffers [P, N2] bf16 (4 tags)
    tr4 = pool("tr4", 3)         # transient ~4KB chunks
    ps = pool("ps", 4, "PSUM")
    pss = pool("pss", 2, "PSUM")
    dpool = pool("dramp", 1, "DRAM")

    dma = nc.sync.dma_start

    # ---------------- inputs / constants
    X1 = bigC.tile([P, 4, N1], FP32, tag="bigC")
    dma(X1[:], dram["x1"].ap().rearrange("(t p) n -> p t n", p=P))

    def load_c(name, shape, dt=FP32):
        t = consts.tile(shape, dt, tag=name)
        src = dram[name].ap()
        if len(shape) == 3:
            src = src.rearrange("(t p) n -> p t n", p=P)
        dma(t[:], src)
        return t

    # order matters: small tiles that gate early work load first
    DW1W = load_c("dw1w", [P, 4, 18])
    DWQW = load_c("dwqw", [P, 2, 18])
    DWOW = load_c("dwow", [P, 2, 9])
    BNPK = load_c("bnpk", [P, 18])
    X2 = big32.tile([P, 2, N2], FP32, tag="big32")
    dma(X2[:], dram["x2"].ap().rearrange("(t p) n -> p t n", p=P))
    WCH = load_c("wch", [P, 4, OUT_CH])
    WKV = load_c("wkv", [P, 4, 2 * OUT_CH], BF16)
    RT16 = load_c("rt16", [P, 8, NS], BF16)
    WQ = load_c("wq", [P, 2, OUT_CH], BF16)
    SELQ1 = load_c("selq1", [P, 2, HEADS], BF16)
    SELQ2 = load_c("selq2", [P, 2, HEADS], BF16)
    GKB = load_c("gkb", [P, OUT_CH], BF16)
    BKB = load_c("bkb", [P, OUT_CH], BF16)
    GQB = load_c("gqb", [P, 2, 1], BF16)
    BQB = load_c("bqb", [P, 2, 1], BF16)
    BVT = load_c("bvt", [P, 2 * HEADS, NS], BF16)
    SELB = load_c("selb", [P, 16, OUT_CH], BF16)
    WOUT = load_c("wout", [P, 2, OUT_CH], BF16)
    WMLP = load_c("wmlp", [P, 2, OUT_CH], BF16)

    IDM = load_c("idm", [P, P], BF16)
    # diag-tap tables for the PE-side taps of DWQ (slots 0-7) and DWO (8-15)
    DIAG = tmpp.tile([P, 16, P], BF16, tag="tmp4", name="dwdiag")
    dma(DIAG[:], dram["dwod"].ap().rearrange("(t p) n -> p t n", p=P)[:, 16:32, :])

    # ---------------- padded raw images + BN stats (Scalar engine, fused)
    XP1 = [work.tile([P, PAD1], BF16, tag=f"XP1_{t}", name=f"XP1_{t}")
           for t in range(4)]
    XP2 = bigA.tile([P, 2, PAD2], BF16, tag="bigA")
    for t in range(4):
        _pad_memsets(nc, XP1[t][:], H1, W1, PW1)
    for t in range(2):
        _pad_memsets(nc, XP2[:, t, :], H2, W2, PW2)

    # layout: 0..7 x1 (S,S2)x4; 8,9 x2 S; 10..13 x2-t0 S2 chunks; 14..17 t1
    ccin = work.tile([P, 18], FP32, tag="ccin")
    trash = work.tile([P, N1], BF16, tag="trash")
    # x1 stats on DVE (idle at the head) so the scalar queue reaches the
    # x2 stats -- which gate the collective -- sooner
    st1 = work.tile([P, 4, 12], FP32, tag="st1")
    ag1 = work.tile([P, 4, 2], FP32, tag="ag1")
    for t in range(4):
        for c in range(2):
            nc.vector.bn_stats(st1[:, t, 6 * c:6 * c + 6], X1[:, t, bass.ts(c, 512)])
        nc.vector.bn_aggr(ag1[:, t, :],
                          st1[:, t, :].rearrange("p (c s) -> p c s", s=6))
        m, v = ag1[:, t, 0:1], ag1[:, t, 1:2]
        S, S2 = ccin[:, 2 * t:2 * t + 1], ccin[:, 2 * t + 1:2 * t + 2]
        nc.vector.tensor_scalar(S, m, float(N1), None, ALU.mult)
        nc.vector.tensor_mul(S2, m, m)
        nc.vector.tensor_add(S2, S2, v)
        nc.vector.tensor_scalar(S2, S2, float(N1), None, ALU.mult)
    for t in range(4):
        nc.scalar.activation(_img(XP1[t][:], PW1)[:, 1:1 + H1, 1:1 + W1],
                             _img(X1[:, t, :], W1), ACTF.Identity, bias=0.0)
    for t in range(2):
        nc.scalar.activation(_img(XP2[:, t, :], PW2)[:, 1:1 + H2, 1:1 + W2],
                             _img(X2[:, t, :], W2), ACTF.Identity, bias=0.0,
                             accum_out=ccin[:, 8 + t:9 + t])
        for c in range(4):
            nc.scalar.activation(trash[:], X2[:, t, bass.ts(c, N1)], ACTF.Square,
                                 accum_out=ccin[:, 10 + 4 * t + c:11 + 4 * t + c])

    # collective input DMA dispatched from the (idle) GpSimd queue so it
    # doesn't sit behind the const loads on the Sync queue
    cc1i = dpool.tile([P, 18], FP32, tag="cc1i")
    cc1o = dpool.tile([P, 18], FP32, tag="cc1o")
    nc.gpsimd.dma_start(cc1i[:], ccin[:])
    nc.gpsimd.collective_compute("AllReduce", ALU.add,
                                 replica_groups=[list(range(NCORES))],
                                 ins=[cc1i.opt()], outs=[cc1o.opt()])
    ccout = work.tile([P, 18], FP32, tag="ccout")

    # ---------------- conv_ch transposed (for the residue, consumed late)
    X1CT = work.tile([P, 8, OUT_CH], BF16, tag="X1CT")
    for m in range(8):
        acc = ps.tile([P, 512], FP32, tag="mm512")
        for kk in range(4):
            nc.tensor.matmul(acc[:, 0:OUT_CH], X1[:, kk, bass.ts(m, P)],
                             WCH[:, kk, :], start=(kk == 0), stop=(kk == 3))
        nc.scalar.copy(X1CT[:, m, :], acc[:, 0:OUT_CH])

    # ---------------- raw depthwise (runs during the AllReduce flight)
    # taps 1-4 of every depthwise tile run on the TensorEngine (idle in this
    # window) as diagonal matmuls; tap 0 is a DVE tensor_scalar; taps 5-8
    # split between DVE STT and Scalar copies depending on who is free
    DW1 = work.tile([P, 4, N1], BF16, tag="DW1")
    tmps1 = [tmpp.tile([P, N2], BF16, tag=f"tmp{j}", name=f"tmp{j}")
             for j in range(5)]
    for t in range(4):
        ns = 0 if t < 2 else 4
        sub = [SubTile(tt, N1) for tt in tmps1]
        _emit_dw_taps(nc, sub, DW1[:, t, :], XP1[t][:], DW1W[:, t, :],
                      H1, W1, PW1, n_scalar=ns)
        _emit_dw_merge(nc, sub, DW1[:, t, :], [], n_scalar=ns)
    DWQ = bigB.tile([P, 2, N2], BF16, tag="bigB")

    # ---------------- BN scale/shift sandwich (DVE reaches this right as
    # the collective returns); ccout fetched from the DVE queue
    nc.scalar.dma_start(ccout[:], cc1o[:])
    bnS = work.tile([P, 6], FP32, tag="bnS")
    bnT = work.tile([P, 6], FP32, tag="bnT")
    mean6 = work.tile([P, 6], FP32, tag="mean6")
    var6 = work.tile([P, 6], FP32, tag="var6")
    s2x2 = work.tile([P, 2], FP32, tag="s2x2")
    for t in range(2):
        nc.vector.tensor_reduce(s2x2[:, t:t + 1],
                                ccout[:, 10 + 4 * t:14 + 4 * t],
                                mybir.AxisListType.X, ALU.add, opt_input=False)
    for t in range(6):
        n = float(B * (N1 if t < 4 else N2))
        if t < 4:
            S, S2 = ccout[:, 2 * t:2 * t + 1], ccout[:, 2 * t + 1:2 * t + 2]
        else:
            S, S2 = ccout[:, 8 + (t - 4):9 + (t - 4)], s2x2[:, t - 4:t - 3]
        m, v = mean6[:, t:t + 1], var6[:, t:t + 1]
        nc.vector.tensor_scalar(m, S, 1.0 / n, None, ALU.mult)
        nc.vector.scalar_tensor_tensor(v, m, -1.0, m, ALU.mult, ALU.mult)
        nc.vector.scalar_tensor_tensor(v, S2, 1.0 / n, v, ALU.mult, ALU.add)
        nc.vector.tensor_scalar(v, v, EPS_BN, None, ALU.add)
    nc.vector.reciprocal(var6[:], var6[:])
    nc.scalar.activation(bnS[:], var6[:], ACTF.Sqrt)
    nc.vector.tensor_mul(bnS[:, 0:4], bnS[:, 0:4], BNPK[:, 0:4])
    nc.vector.tensor_mul(bnS[:, 4:6], bnS[:, 4:6], BNPK[:, 8:10])
    nc.vector.tensor_mul(mean6[:], mean6[:], bnS[:])
    nc.vector.tensor_sub(bnT[:, 0:4], BNPK[:, 4:8], mean6[:, 0:4])
    nc.vector.tensor_sub(bnT[:, 4:6], BNPK[:, 10:12], mean6[:, 4:6])

    # edge scalars: ew0 = t*W9; ewe = -t*[r0,r2,c0,c2]; ewc = t*corners
    negT = work.tile([P, 6], FP32, tag="negT")
    nc.vector.tensor_scalar(negT[:], bnT[:], -1.0, None, ALU.mult)
    EW01 = work.tile([P, 4, 1], FP32, tag="EW01")
    EWE1 = work.tile([P, 4, 4], FP32, tag="EWE1")
    EWC1 = work.tile([P, 4, 4], FP32, tag="EWC1")
    EW0Q = work.tile([P, 2, 1], FP32, tag="EW0Q")
    EWEQ = work.tile([P, 2, 4], FP32, tag="EWEQ")
    EWCQ = work.tile([P, 2, 4], FP32, tag="EWCQ")
    for t in range(4):
        nc.vector.tensor_scalar(EW01[:, t, :], DW1W[:, t, 9:10],
                                bnT[:, t:t + 1], None, ALU.mult)
        nc.vector.tensor_scalar(EWE1[:, t, :], DW1W[:, t, 10:14],
                                negT[:, t:t + 1], None, ALU.mult)
        nc.vector.tensor_scalar(EWC1[:, t, :], DW1W[:, t, 14:18],
                                bnT[:, t:t + 1], None, ALU.mult)
    for t in range(2):
        nc.vector.tensor_scalar(EW0Q[:, t, :], DWQW[:, t, 9:10],
                                bnT[:, 4 + t:5 + t], None, ALU.mult)
        nc.vector.tensor_scalar(EWEQ[:, t, :], DWQW[:, t, 10:14],
                                negT[:, 4 + t:5 + t], None, ALU.mult)
        nc.vector.tensor_scalar(EWCQ[:, t, :], DWQW[:, t, 14:18],
                                bnT[:, 4 + t:5 + t], None, ALU.mult)

    # first DWQ tile (scalar tap copies already queued ahead of the affines)
    paccq0 = _emit_dw_pe_taps(nc, ps, DIAG, 0, XP2[:, 0, :],
                              H2, W2, PW2, "dqp0_")
    _emit_dw_taps(nc, tmps1, DWQ[:, 0, :], XP2[:, 0, :], DWQW[:, 0, :],
                  H2, W2, PW2, n_scalar=3, pe=True)
    _emit_dw_merge(nc, tmps1, DWQ[:, 0, :], paccq0, n_scalar=3)

    # apply BN affine to DW1 (Scalar engine only), then kv pointwise can start
    for t in range(4):
        _emit_dw_affine(nc, DW1[:, t, :], bnS[:, t:t + 1], EW01[:, t, :],
                        EWE1[:, t, :], EWC1[:, t, :], H1, W1)

    # ---------------- kv pointwise (pixel-major out)
    KVT = big16.tile([P, 8, 2 * OUT_CH], BF16, tag="big16")
    for m in range(8):
        acc = ps.tile([P, 512], FP32, tag="mm512")
        for kk in range(4):
            nc.tensor.matmul(acc[:], DW1[:, kk, bass.ts(m, P)], WKV[:, kk, :],
                             start=(kk == 0), stop=(kk == 3))
        nc.vector.tensor_copy(KVT[:, m, :], acc[:])

    # second DWQ tile + both affines
    paccq1 = _emit_dw_pe_taps(nc, ps, DIAG, 4, XP2[:, 1, :],
                              H2, W2, PW2, "dqp1_")
    _emit_dw_taps(nc, tmps1, DWQ[:, 1, :], XP2[:, 1, :], DWQW[:, 1, :],
                  H2, W2, PW2, n_scalar=3, pe=True)
    _emit_dw_merge(nc, tmps1, DWQ[:, 1, :], paccq1, n_scalar=3)
    for t in range(2):
        _emit_dw_affine(nc, DWQ[:, t, :], bnS[:, 4 + t:5 + t], EW0Q[:, t, :],
                        EWEQ[:, t, :], EWCQ[:, t, :], H2, W2)

    # resize 32->16: kvsT = RT16^T @ KVT  [256 smallpix, 512]
    KVS = []
    for mm in range(2):
        acc = pss.tile([P, 512], FP32, tag="psmall")
        for kk in range(8):
            nc.tensor.matmul(acc[:], RT16[:, kk, bass.ts(mm, P)], KVT[:, kk, :],
                             start=(kk == 0), stop=(kk == 7))
        KVS.append(acc)

    # LN-k + evac k' ; v' plain evac (bf16)
    KP = work.tile([P, 2, OUT_CH], BF16, tag="KP")
    VP = work.tile([P, 2, OUT_CH], BF16, tag="VP")
    ksq = work.tile([P, OUT_CH], FP32, tag="ksq")
    ksum = work.tile([P, HEADS], FP32, tag="ksum")
    km = work.tile([P, HEADS], FP32, tag="km")
    krs = work.tile([P, HEADS], FP32, tag="krs")
    kfp = work.tile([P, OUT_CH], FP32, tag="kfp")
    for mm in range(2):
        k_ap = KVS[mm][:, 0:OUT_CH].rearrange("p (h d) -> p h d", d=DIM_HEAD)
        nc.vector.tensor_reduce(ksum[:], k_ap, mybir.AxisListType.X, ALU.add,
                                opt_input=False)
        nc.scalar.activation(ksq[:], KVS[mm][:, 0:OUT_CH], ACTF.Square)
        nc.vector.tensor_reduce(krs[:], ksq[:].rearrange("p (h d) -> p h d",
                                                         d=DIM_HEAD),
                                mybir.AxisListType.X, ALU.add, opt_input=False)
        nc.vector.scalar_tensor_tensor(km[:], ksum[:], -1.0 / DIM_HEAD, ksum[:],
                                       ALU.mult, ALU.mult)
        nc.vector.tensor_add(krs[:], krs[:], km[:])
        nc.vector.tensor_scalar(krs[:], krs[:], DIM_HEAD * EPS_LN, None, ALU.add)
        nc.vector.reciprocal(krs[:], krs[:])
        nc.scalar.activation(krs[:], krs[:], ACTF.Sqrt, scale=float(DIM_HEAD))
        nc.vector.tensor_scalar(km[:], ksum[:], 1.0 / DIM_HEAD, None, ALU.mult)
        kb = km[:].unsqueeze(2).broadcast_to([P, HEADS, DIM_HEAD])
        rb = krs[:].unsqueeze(2).broadcast_to([P, HEADS, DIM_HEAD])
        t1 = kfp[:].rearrange("p (h d) -> p h d", d=DIM_HEAD)
        nc.vector.tensor_sub(t1, k_ap, kb)
        nc.vector.tensor_mul(t1, t1, rb)
        nc.vector.tensor_mul(kfp[:], kfp[:], GKB[:])
        nc.vector.tensor_add(KP[:, mm, :], kfp[:], BKB[:])
        nc.vector.tensor_copy(VP[:, mm, :], KVS[mm][:, OUT_CH:2 * OUT_CH])

    # A = K'^T V' / 32 : diagonal head blocks packed block-diagonal
    BD = work.tile([P, 2, P], BF16, tag="BD")
    nc.gpsimd.memset(BD[:], 0.0)
    for mo in range(2):
        acc = pss.tile([P, 512], FP32, tag="psmall")
        for kk in range(2):
            nc.tensor.matmul(acc[:, 0:OUT_CH], KP[:, kk, bass.ts(mo, P)],
                             VP[:, kk, :], start=(kk == 0), stop=(kk == 1))
        for hh in range(4):
            h = mo * 4 + hh
            nc.scalar.activation(BD[bass.ds(32 * hh, 32), mo, bass.ds(32 * hh, 32)],
                                 acc[bass.ds(32 * hh, 32), bass.ds(32 * h, 32)],
                                 ACTF.Copy, scale=1.0 / DIM_HEAD)

    # Bb = BD @ b_q, Gg = BD @ g (per-channel consts for deferred LN-q)
    BbGg = work.tile([P, 2, 2], FP32, tag="BbGg")   # [:, pk, 0]=Bb, 1=-Gg
    for pk in range(2):
        acc = pss.tile([P, 512], FP32, tag="psmall")
        nc.tensor.matmul(acc[:, 0:1], BD[:, pk, :], BQB[:, pk, :],
                         start=True, stop=True)
        nc.tensor.matmul(acc[:, 1:2], BD[:, pk, :], GQB[:, pk, :],
                         start=True, stop=True)
        nc.scalar.copy(BbGg[:, pk, 0:1], acc[:, 0:1])
        nc.scalar.activation(BbGg[:, pk, 1:2], acc[:, 1:2], ACTF.Identity,
                             bias=0.0, scale=-1.0)

    # BV[(h,d'), is] = (v'^T bias_small_h)/32 via full-M matmul + row extract
    BVC = work.tile([P, 2, NS], BF16, tag="BVC")
    for h in range(HEADS):
        mo, hh = h // 4, h % 4
        acc = pss.tile([P, 512], FP32, tag="psmall")
        for kk in range(2):
            nc.tensor.matmul(acc[:, 0:NS], VP[:, kk, bass.ts(mo, P)],
                             BVT[:, 2 * h + kk, :], start=(kk == 0), stop=(kk == 1))
        nc.scalar.activation(BVC[bass.ds(32 * hh, 32), mo, :],
                             acc[bass.ds(32 * hh, 32), 0:NS],
                             ACTF.Copy, scale=1.0 / DIM_HEAD)
    # expand along x: BVX[c, ys*64 + x] = BVC[c, ys*16 + x//4]; then += Bb
    BVX = work.tile([P, 2, R * W2], BF16, tag="BVX")
    for mo in range(2):
        nc.vector.tensor_copy(
            BVX[:, mo, :].rearrange("p (ys xs xr) -> p ys xs xr", xs=R, xr=4),
            BVC[:, mo, :].rearrange("p (ys xs) -> p ys xs", xs=R)
            .unsqueeze(3).broadcast_to([P, R, R, 4]))
        nc.vector.tensor_scalar(BVX[:, mo, :], BVX[:, mo, :],
                                BbGg[:, mo, 0:1], None, ALU.add)

    # ---------------- q pointwise (g-folded) + LN-q stats
    Q = bigC.tile([P, 2, N2], BF16, tag="bigC")     # reuses X1 slot
    QSP = work.tile([P, 2, NS], FP32, tag="QSP")   # [(h*16+blk), (qs|q2s), 256]
    for nn in range(8):
        q2c = tr4.tile([P, 2, 512], BF16, tag="tr4")
        for mm in range(2):
            acc = ps.tile([P, 512], FP32, tag="mm512")
            for kk in range(2):
                nc.tensor.matmul(acc[:], WQ[:, kk, bass.ts(mm, P)],
                                 DWQ[:, kk, bass.ts(nn, 512)],
                                 start=(kk == 0), stop=(kk == 1))
            nc.scalar.copy(Q[:, mm, bass.ts(nn, 512)], acc[:])
            nc.vector.tensor_mul(q2c[:, mm, :], Q[:, mm, bass.ts(nn, 512)],
                                 Q[:, mm, bass.ts(nn, 512)])
        for s in range(2):
            sacc = pss.tile([P, 512], FP32, tag="psmall")
            SEL = SELQ1 if s == 0 else SELQ2
            for mm in range(2):
                rhs = Q[:, mm, bass.ts(nn, 512)] if s == 0 else q2c[:, mm, :]
                nc.tensor.matmul(sacc[0:HEADS, :], SEL[:, mm, :], rhs,
                                 start=(mm == 0), stop=(mm == 1))
            qsc = tr4.tile([HEADS, 512], FP32, tag="tr4")
            nc.vector.tensor_copy(qsc[:], sacc[0:HEADS, :])
            # relayout rows: row 16nn + 2h + b  <->  (blk = 2nn+b, h)
            dma(QSP[bass.ds(16 * nn, 16), s, :],
                qsc[:].rearrange("h (b f) -> h b f", f=NS))

    # rs | mrs  (bf16, packed for the broadcast matmul)
    RSP = work.tile([P, 2, NS], BF16, tag="RSP")
    numt = work.tile([P, NS], FP32, tag="numt")
    qsv, q2v = QSP[:, 0, :], QSP[:, 1, :]
    nc.vector.scalar_tensor_tensor(numt[:], qsv, -1.0 / DIM_HEAD, qsv, ALU.mult, ALU.mult)
    nc.vector.tensor_add(numt[:], numt[:], q2v)
    nc.vector.tensor_scalar(numt[:], numt[:], DIM_HEAD * EPS_LN, None, ALU.add)
    nc.vector.reciprocal(numt[:], numt[:])
    nc.scalar.activation(RSP[:, 0, :], numt[:], ACTF.Sqrt, scale=float(DIM_HEAD))
    nc.vector.scalar_tensor_tensor(RSP[:, 1, :], qsv, 1.0 / DIM_HEAD, RSP[:, 0, :],
                                   ALU.mult, ALU.mult)

    # ---------------- per-256-pixel block: broadcast stats, QA matmul on
    # g-folded Q, deferred LN affine on the output, add bias map, write OPAD
    OPAD = bigA.tile([P, 2, PAD2], BF16, tag="bigA")   # reuses XP2 slot
    for t in range(2):
        _pad_memsets(nc, OPAD[:, t, :], H2, W2, PW2)
    rsp_flat = RSP[:].rearrange("p s f -> p (s f)")
    for blk in range(16):
        rsb = tr4.tile([P, 2, 2, NS], BF16, tag="tr4")
        for mm in range(2):
            bacc = pss.tile([P, 512], FP32, tag="psmall")
            nc.tensor.matmul(bacc[:], SELB[:, blk, bass.ts(mm, P)], rsp_flat,
                             start=True, stop=True)
            nc.scalar.copy(rsb[:, mm, :, :],
                           bacc[:].rearrange("p (s f) -> p s f", f=NS))
        for pk in range(2):
            acc = ps.tile([P, 512], FP32, tag="mm512")
            nc.tensor.matmul(acc[:, 0:NS], BD[:, pk, :],
                             Q[:, pk, bass.ds(blk * NS, NS)],
                             start=True, stop=True)
            tmpo = tr4.tile([P, NS], BF16, tag="tr4b")
            nc.vector.tensor_mul(tmpo[:], acc[:, 0:NS], rsb[:, pk, 0, :])
            nc.vector.scalar_tensor_tensor(tmpo[:], rsb[:, pk, 1, :],
                                           BbGg[:, pk, 1:2], tmpo[:],
                                           ALU.mult, ALU.add)
            # rows 4*blk .. 4*blk+4 of the 64x64 image; ys = blk
            dst = _img(OPAD[:, pk, :], PW2)[:, 1 + 4 * blk:5 + 4 * blk, 1:1 + W2]
            bv = BVX[:, pk, bass.ds(blk * W2, W2)].unsqueeze(1) \
                .broadcast_to([P, 4, W2])
            nc.gpsimd.tensor_add(dst,
                                 tmpo[:].rearrange("p (yr w) -> p yr w", w=W2),
                                 bv)

    # ---------------- to_out depthwise + pointwise + residue in PSUM
    # taps 1-4 run on the (otherwise idle) TensorEngine as diagonal-weight
    # matmuls PSUM-accumulated over shifted OPAD views; tap 0 is a DVE
    # tensor_scalar; taps 5-8 are scalar-engine copies merged on DVE.
    DWO = bigB.tile([P, 2, N2], BF16, tag="bigB")   # reuses DWQ slot
    for t in range(2):
        pacco = _emit_dw_pe_taps(nc, ps, DIAG, 8 + 4 * t, OPAD[:, t, :],
                                 H2, W2, PW2, f"dop{t}_")
        _emit_dw_taps(nc, tmps1, DWO[:, t, :], OPAD[:, t, :], DWOW[:, t, :],
                      H2, W2, PW2, n_scalar=4, pe=True)
        _emit_dw_merge(nc, tmps1, DWO[:, t, :], pacco, n_scalar=4)
    # residue-resize table loads into the freed tap-tmp slabs (3 chunks of 8)
    r64src = dram["r64c"].ap().rearrange("(t p) n -> p t n", p=P)
    R64T = []
    for c in range(3):
        lo, hi = 8 * c, min(8 * c + 8, _N_R64_SLOTS)
        rc = tmpp.tile([P, hi - lo, 512], BF16, tag=f"tmp{c}", name=f"r64t{c}")
        dma(rc[:], r64src[:, lo:hi, :])
        R64T.append(rc)
    OSB = big32.tile([P, 2, N2], BF16, tag="big32")   # reuses X2 slot
    # OSB evac (+conv_ch bias) and BN2 stats on DVE -- it is idle during
    # the WOUT matmul phase while the scalar engine was the pacer before
    st2 = work.tile([P, 2, 48], FP32, tag="st2")
    ag2 = work.tile([P, 2, 2], FP32, tag="ag2")
    slot = 0
    for nn in range(8):
        used = _R64_KTILES[nn]
        for mm in range(2):
            acc = ps.tile([P, 512], FP32, tag="mm512")
            for kk in range(2):
                nc.tensor.matmul(acc[:], WOUT[:, kk, bass.ts(mm, P)],
                                 DWO[:, kk, bass.ts(nn, 512)],
                                 start=(kk == 0), stop=False)
            for i, kk in enumerate(used):
                s = slot + i
                nc.tensor.matmul(acc[:], X1CT[:, kk, bass.ts(mm, P)],
                                 R64T[s // 8][:, s % 8, :],
                                 start=False, stop=(i == len(used) - 1))
            nc.vector.tensor_scalar(OSB[:, mm, bass.ts(nn, 512)], acc[:],
                                    BNPK[:, 16 + mm:17 + mm], None, ALU.add)
            nc.vector.bn_stats(st2[:, mm, 6 * nn:6 * nn + 6],
                               OSB[:, mm, bass.ts(nn, 512)])
        slot += len(used)

    # ---------------- BN2 (norm2) stats reduce + AllReduce
    cc2s = work.tile([P, 4], FP32, tag="cc2s")
    for t in range(2):
        nc.vector.bn_aggr(ag2[:, t, :],
                          st2[:, t, :].rearrange("p (c s) -> p c s", s=6))
        m, v = ag2[:, t, 0:1], ag2[:, t, 1:2]
        S, S2 = cc2s[:, 2 * t:2 * t + 1], cc2s[:, 2 * t + 1:2 * t + 2]
        nc.vector.tensor_scalar(S, m, float(N2), None, ALU.mult)
        nc.vector.tensor_mul(S2, m, m)
        nc.vector.tensor_add(S2, S2, v)
        nc.vector.tensor_scalar(S2, S2, float(N2), None, ALU.mult)
    cc2i = dpool.tile([P, 4], FP32, tag="cc2i")
    cc2o = dpool.tile([P, 4], FP32, tag="cc2o")
    nc.gpsimd.dma_start(cc2i[:], cc2s[:])
    nc.gpsimd.collective_compute("AllReduce", ALU.add,
                                 replica_groups=[list(range(NCORES))],
                                 ins=[cc2i.opt()], outs=[cc2o.opt()])
    cc2r = work.tile([P, 4], FP32, tag="cc2r")
    nc.scalar.dma_start(cc2r[:], cc2o[:])
    bn3S = work.tile([P, 2], FP32, tag="bn3S")
    bn3T = work.tile([P, 2], FP32, tag="bn3T")
    m3 = work.tile([P, 2], FP32, tag="m3")
    v3 = work.tile([P, 2], FP32, tag="v3")
    nB = float(B * N2)
    for t in range(2):
        S, S2 = cc2r[:, 2 * t:2 * t + 1], cc2r[:, 2 * t + 1:2 * t + 2]
        nc.vector.tensor_scalar(m3[:, t:t + 1], S, 1.0 / nB, None, ALU.mult)
        nc.vector.scalar_tensor_tensor(v3[:, t:t + 1], m3[:, t:t + 1], -1.0,
                                       m3[:, t:t + 1], ALU.mult, ALU.mult)
        nc.vector.scalar_tensor_tensor(v3[:, t:t + 1], S2, 1.0 / nB,
                                       v3[:, t:t + 1], ALU.mult, ALU.add)
        nc.vector.tensor_scalar(v3[:, t:t + 1], v3[:, t:t + 1], EPS_BN, None, ALU.add)
    nc.vector.reciprocal(v3[:], v3[:])
    nc.scalar.activation(bn3S[:], v3[:], ACTF.Sqrt)
    nc.vector.tensor_mul(bn3S[:], bn3S[:], BNPK[:, 12:14])
    nc.vector.tensor_mul(m3[:], m3[:], bn3S[:])
    nc.vector.tensor_sub(bn3T[:], BNPK[:, 14:16], m3[:])

    # ---------------- relu(bn) + mlp + final residual -> out
    # relu is chunked into the mlp loop so the first matmul starts ~6us
    # earlier after the BN2 collective returns
    RELU = bigC.tile([P, 2, N2], BF16, tag="bigC")   # reuses Q slot
    out_ap = out_d.ap().rearrange("(t p) n -> p t n", p=P)
    for nn in range(8):
        for t in range(2):
            nc.scalar.activation(RELU[:, t, bass.ts(nn, 512)],
                                 OSB[:, t, bass.ts(nn, 512)], ACTF.Relu,
                                 bias=bn3T[:, t:t + 1], scale=bn3S[:, t:t + 1])
        for mm in range(2):
            acc = ps.tile([P, 512], FP32, tag="mm512")
            for kk in range(2):
                nc.tensor.matmul(acc[:], WMLP[:, kk, bass.ts(mm, P)],
                                 RELU[:, kk, bass.ts(nn, 512)],
                                 start=(kk == 0), stop=(kk == 1))
            fin = tr4.tile([P, 512], FP32, tag="tr4")
            nc.vector.tensor_add(fin[:], acc[:], OSB[:, mm, bass.ts(nn, 512)])
            dma(out_ap[:, mm, bass.ts(nn, 512)], fin[:])

    ctx.close()


class SubTile:
    """View adapter: presents the first n columns of a tile as a tile."""
    def __init__(self, t, n):
        self._t = t
        self._n = n

    def __getitem__(self, key):
        return self._t[:, 0:self._n]


def _build_program():
    nc = bacc.Bacc("TRN2", target_bir_lowering=False, debug=False,
                   num_devices=NCORES)
    dram = {}

    def din(name, shape, dt=FP32):
        dram[name] = nc.dram_tensor(name, list(shape), dt, kind="ExternalInput")

    din("x1", (IN_CH, N1)); din("x2", (OUT_CH, N2))
    din("wch", (IN_CH, OUT_CH)); din("wkv", (IN_CH, 2 * OUT_CH), BF16)
    din("wq", (OUT_CH, OUT_CH), BF16); din("wout", (OUT_CH, OUT_CH), BF16)
    din("wmlp", (OUT_CH, OUT_CH), BF16)
    din("dw1w", (IN_CH, 18)); din("dwqw", (OUT_CH, 18)); din("dwow", (OUT_CH, 9))
    din("dwod", (32 * P, P), BF16)
    din("idm", (P, P), BF16)
    din("rt16", (N1, NS), BF16); din("r64c", (_N_R64_SLOTS * P, 512), BF16)
    din("selq1", (OUT_CH, HEADS), BF16); din("selq2", (OUT_CH, HEADS), BF16)
    din("selb", (16 * P, OUT_CH), BF16)
    din("bvt", (HEADS * NS, NS), BF16)
    din("gkb", (P, OUT_CH), BF16); din("bkb", (P, OUT_CH), BF16)
    din("gqb", (OUT_CH, 1), BF16); din("bqb", (OUT_CH, 1), BF16)
    din("bnpk", (P, 18))
    out_d = nc.dram_tensor("out", [OUT_CH, N2], FP32, kind="ExternalOutput")

    with tile.TileContext(nc) as tc:
        _emit(nc, tc, dram, out_d)
    nc.compile()
    return nc


# ------------------------------------------------------------------- run layer

_CACHE = {}


def _get_program():
    if "nc" not in _CACHE:
        _CACHE["nc"] = _build_program()
    return _CACHE["nc"]


def kernel(**inputs):
    nc = _get_program()
    shared = _host_prep(inputs)
    x1 = np.ascontiguousarray(np.asarray(inputs["x1"], np.float32).reshape(B, IN_CH, N1))
    x2 = np.ascontiguousarray(np.asarray(inputs["x2"], np.float32).reshape(B, OUT_CH, N2))
    in_maps = [dict(shared, x1=x1[b], x2=x2[b]) for b in range(B)]
    res = run_bass_kernel_spmd(nc, in_maps, core_ids=list(range(NCORES)))
    out = np.stack([np.asarray(res.results[b]["out"], np.float32)
                    .reshape(OUT_CH, H2, W2) for b in range(B)])
    return out
